# revision 21
# baseline (speedup 1.0000x reference)
"""Trainium2 Bass kernel for nn_Attention (channel attention, XCA-style) v3.

Sharding: 8 cores = (batch b=core//2) x (image half = core%2, 64 rows + halo).
Cross-core: AllGather of tiny gram stats over core pairs + local add.

Fully interleaved single pass (one 8-bank PSUM pool):
  per 512-px chunk: conv q,k (fp8 DoubleRow) + conv v (fp8 DR with
  value/residual/weight-residual passes) -> padded fp8/bf16 t-buffers;
  per 16-row stripe: DW q,k via diag tap-pair DR matmuls -> bf16 stripes ->
  batched DMA-xbar transpose -> gram matmuls + Sc Square norms; DW v:
  128-chan block via DR with (value,residual) pair + tap-paired
  weight-error correction, 64-chan block on DVE (bf16 exact).
  AllGather([192,26]) pairs -> local add -> softmax glue ->
  fused (Wp @ blockdiag(attn)) @ v -> bf16 out.
"""

import sys
import numpy as np

sys.path.insert(0, "/opt/trn_rl_repo")

import contextlib  # noqa: E402

import ml_dtypes  # noqa: E402

from concourse import bass, bacc, tile, mybir  # noqa: E402
from concourse import bass_utils  # noqa: E402
from concourse.ap import AP  # noqa: E402

F32 = mybir.dt.float32
BF16 = mybir.dt.bfloat16
FP8 = mybir.dt.float8e4
ALU = mybir.AluOpType
ACTF = mybir.ActivationFunctionType
AX = mybir.AxisListType
BF16NP = ml_dtypes.bfloat16
FP8NP = ml_dtypes.float8_e4m3
DR = mybir.MatmulPerfMode.DoubleRow

C = 192
HEADS = 8
CH = 24
W = 128
HOUT = 64
HIN = HOUT + 2
PXIN = HIN * W            # 8448
PXOUT = HOUT * W          # 8192
WS = 130                  # padded row stride in t buffers
LT = HIN * WS             # 8580
RS = 16                   # stripe out-rows
NS = HOUT // RS           # 4 stripes
MM = 512

TAPS = [(dy, dx) for dy in (0, 1, 2) for dx in (0, 1, 2)]
TOFF = [dy * WS + dx for dy, dx in TAPS]
PAIRS = [(0, 1), (2, 3), (4, 5), (6, 7), (8, 8)]

_CACHE = {}


def _chunks(total, step):
    out, s = [], 0
    while s < total:
        out.append((s, min(step, total - s)))
        s += step
    return out


def _mk(base_ap, off, dims):
    ap0 = [list(base_ap.ap[0])]
    return AP(base_ap.tensor, base_ap.offset + off,
              ap0 + [list(d) for d in dims])


def build_program():
    nc = bacc.Bacc("TRN2", target_bir_lowering=False, debug=False,
                   enable_asserts=False, num_devices=8)
    io = {}
    io["y8"] = nc.dram_tensor("y8", [128, 2 * PXIN], FP8,
                              kind="ExternalInput").ap()
    io["x4"] = nc.dram_tensor("x4", [128, 4 * PXIN], FP8,
                              kind="ExternalInput").ap()
    io["wqk8"] = nc.dram_tensor("wqk8", [128, 2 * 448], FP8,
                                kind="ExternalInput").ap()
    io["wv8"] = nc.dram_tensor("wv8", [128, 2 * 192], FP8,
                               kind="ExternalInput").ap()
    io["wve8"] = nc.dram_tensor("wve8", [128, 2 * 192], FP8,
                                kind="ExternalInput").ap()
    io["dgqk"] = nc.dram_tensor("dgqk", [128, 3 * 5 * 256], FP8,
                                kind="ExternalInput").ap()
    io["dgva"] = nc.dram_tensor("dgva", [128, 9 * 128], BF16,
                                kind="ExternalInput").ap()
    io["dvb"] = nc.dram_tensor("dvb", [64, 9], F32,
                               kind="ExternalInput").ap()
    io["dgvb16"] = nc.dram_tensor("dgvb16", [64, 9 * 64], BF16,
                                  kind="ExternalInput").ap()
    io["wpa"] = nc.dram_tensor("wpa", [128, C], BF16,
                               kind="ExternalInput").ap()
    io["wpb"] = nc.dram_tensor("wpb", [64, C], BF16,
                               kind="ExternalInput").ap()
    io["em"] = nc.dram_tensor("em", [HEADS, C], BF16,
                              kind="ExternalInput").ap()
    io["emba"] = nc.dram_tensor("emba", [128, C], BF16,
                                kind="ExternalInput").ap()
    io["embb"] = nc.dram_tensor("embb", [64, C], BF16,
                                kind="ExternalInput").ap()
    io["eye"] = nc.dram_tensor("eye", [128, 128], F32,
                               kind="ExternalInput").ap()
    io["tmpq"] = nc.dram_tensor("tmpq", [128, 2], F32,
                                kind="ExternalInput").ap()
    io["outp"] = nc.dram_tensor("outp", [C, PXOUT], BF16,
                                kind="ExternalOutput").ap()

    with tile.TileContext(nc) as tc, contextlib.ExitStack() as es:
        _emit(nc, tc, io, es)
    nc.compile()
    return nc


def _emit(nc, tc, io, es):
    # ---------------- pools & persistent tiles ------------------------
    wpool = es.enter_context(tc.tile_pool(name="w", bufs=1))
    dgqk = wpool.tile([128, 3 * 5 * 256], FP8, tag="dgqk")
    dgva = wpool.tile([128, 9 * 128], BF16, tag="dgva")
    dvb_t = wpool.tile([64, 9], F32, tag="dvb")
    dgvb16 = wpool.tile([64, 9 * 64], BF16, tag="dgvb16")
    wpa = wpool.tile([128, C], BF16, tag="wpa")
    wpb = wpool.tile([64, C], BF16, tag="wpb")
    em_t = wpool.tile([HEADS, C], BF16, tag="em")
    emba = wpool.tile([128, C], BF16, tag="emba")
    embb = wpool.tile([64, C], BF16, tag="embb")
    eye_t = wpool.tile([128, 128], F32, tag="eye")
    tmpq_t = wpool.tile([128, 2], F32, tag="tmpq")

    tpool = es.enter_context(tc.tile_pool(name="t", bufs=1))
    t_blk = [tpool.tile([128, LT], FP8, tag=f"t{b}", name=f"t{b}")
             for b in range(3)]
    tva16 = tpool.tile([128, LT], BF16, tag="tva16")
    tvb16 = tpool.tile([64, LT], BF16, tag="tvb16")

    dwp = es.enter_context(tc.tile_pool(name="dw", bufs=2))
    stkp = es.enter_context(tc.tile_pool(name="stk", bufs=1))
    vp = es.enter_context(tc.tile_pool(name="v", bufs=1))
    v16a = vp.tile([128, PXOUT], BF16, tag="v16a")
    v16b = vp.tile([64, PXOUT], BF16, tag="v16b")
    vbtmp = vp.tile([64, RS * W], BF16, tag="vbtmp")
    small = es.enter_context(tc.tile_pool(name="sm", bufs=1))
    drm = es.enter_context(tc.tile_pool(name="drm", bufs=1, space="DRAM"))

    ines = contextlib.ExitStack()
    inpool = ines.enter_context(tc.tile_pool(name="inp", bufs=1))
    y8 = inpool.tile([128, 2 * PXIN], FP8, tag="y8")
    x4 = inpool.tile([128, 4 * PXIN], FP8, tag="x4")
    wqk8 = inpool.tile([128, 2 * 448], FP8, tag="wqk8")
    wv8 = inpool.tile([128, 2 * 192], FP8, tag="wv8")
    wve8 = inpool.tile([128, 2 * 192], FP8, tag="wve8")

    # conv weights first on the sync queue (gate the first matmuls)
    for nm, t in (("wqk8", wqk8), ("wv8", wv8), ("wve8", wve8),
                  ("dgqk", dgqk), ("dgva", dgva), ("dvb", dvb_t),
                  ("dgvb16", dgvb16)):
        nc.sync.dma_start(t[:], io[nm])
    # inputs: split by pixel range; value-halves first (gate conv qk),
    # x residual halves last (only needed by the later v-phase)
    for part in range(4):
        a, b = part * PXIN // 4, (part + 1) * PXIN // 4
        nc.gpsimd.dma_start(y8[:, a:b], io["y8"][:, a:b])
        nc.gpsimd.dma_start(y8[:, PXIN + a:PXIN + b],
                            io["y8"][:, PXIN + a:PXIN + b])
        for sec in (0, 2):
            o = sec * PXIN
            nc.gpsimd.dma_start(x4[:, o + a:o + b], io["x4"][:, o + a:o + b])
    for part in range(4):
        a, b = part * PXIN // 4, (part + 1) * PXIN // 4
        for sec in (1, 3):
            o = sec * PXIN
            nc.gpsimd.dma_start(x4[:, o + a:o + b], io["x4"][:, o + a:o + b])
    for nm, t in (("wpa", wpa), ("wpb", wpb), ("em", em_t), ("eye", eye_t),
                  ("emba", emba), ("embb", embb), ("tmpq", tmpq_t)):
        nc.sync.dma_start(t[:], io[nm])

    # pad-column zeroing
    for t in t_blk:
        v = t[:].rearrange("p (r w) -> p r w", w=WS)
        nc.vector.memset(v[:, :, 0:1], 0.0)
        nc.vector.memset(v[:, :, 129:130], 0.0)
    v = tva16[:].rearrange("p (r w) -> p r w", w=WS)
    nc.vector.memset(v[:, :, 0:1], 0.0)
    nc.vector.memset(v[:, :, 129:130], 0.0)
    v = tvb16[:].rearrange("p (r w) -> p r w", w=WS)
    nc.vector.memset(v[:, :, 0:1], 0.0)
    nc.vector.memset(v[:, :, 129:130], 0.0)

    w2v = wqk8[:].rearrange("p (two m) -> p two m", two=2)
    wv8v = wv8[:].rearrange("p (two m) -> p two m", two=2)
    wve8v = wve8[:].rearrange("p (two m) -> p two m", two=2)

    def y8rhs(n0, n):
        return _mk(y8[:], n0, [[PXIN, 2], [1, n]])

    def x8rhs(n0, n, res=0):
        return _mk(x4[:], res * PXIN + n0, [[2 * PXIN, 2], [1, n]])

    cchunks = _chunks(PXIN, MM)
    gab_sb = small.tile([128, 640], F32, tag="gabsb")
    qn_part = small.tile([128, 3 * NS + 4], F32, tag="qnp")
    junk = small.tile([128, RS * W], BF16, tag="junk")
    tva3 = tva16[:].rearrange("p (r w) -> p r w", w=WS)
    tvb3 = tvb16[:].rearrange("p (r w) -> p r w", w=WS)

    # ============ fused pass: conv qk+v, DW, gram =====================
    with tc.tile_pool(name="pa", bufs=1, space="PSUM") as pa:
        gAB = pa.tile([128, 640], F32, tag="gAB")
        g1v = gAB[:, 0:384].rearrange("p (r c) -> p r c", c=128)
        g2v = gAB[:, 384:640].rearrange("p (r c) -> p r c", c=128)

        def conv_qk(ci):
            n0, n = cchunks[ci]
            r0, nr = n0 // W, n // W
            ps0 = pa.tile([128, MM], F32, tag="cv0", name=f"cv0_{ci}")
            ps1 = pa.tile([128, MM], F32, tag="cv1", name=f"cv1_{ci}")
            ps2 = pa.tile([128, MM], F32, tag="cv2", name=f"cv2_{ci}",
                          bufs=2)
            nc.tensor.matmul(ps0[:, 0:n], w2v[:, :, 0:128], y8rhs(n0, n),
                             start=True, stop=True, perf_mode=DR)
            # t1 = [k0:64 @ parts 0:64 ; q128:192 @ parts 64:128]
            nc.tensor.matmul(ps1[:, 0:n], w2v[:, :, 128:256], y8rhs(n0, n),
                             start=True, stop=False, perf_mode=DR)
            nc.tensor.matmul(ps1[0:64, 0:n], w2v[:, :, 256:320],
                             x8rhs(n0, n), start=False, stop=True,
                             perf_mode=DR, skip_group_check=True)
            nc.tensor.matmul(ps2[:, 0:n], w2v[:, :, 320:448], x8rhs(n0, n),
                             start=True, stop=True, perf_mode=DR)
            for b, ps, eng in ((0, ps0, 0), (1, ps1, 1), (2, ps2, 1)):
                dst = t_blk[b][:].rearrange("p (r w) -> p r w", w=WS)
                src = ps[:, 0:n].rearrange("p (r w) -> p r w", w=W)
                if eng == 0:
                    nc.scalar.copy(dst[:, r0:r0 + nr, 1:129], src)
                else:
                    nc.vector.tensor_copy(dst[:, r0:r0 + nr, 1:129], src)

        def conv_v(ci):
            n0, n = cchunks[ci]
            r0, nr = n0 // W, n // W
            psa = pa.tile([128, MM], F32, tag="cv0", name=f"cva_{ci}")
            for m0, m1 in ((0, 128),):
                nc.tensor.matmul(psa[:, 0:n], wv8v[:, :, m0:m1],
                                 x8rhs(n0, n), start=True, stop=False,
                                 perf_mode=DR)
                nc.tensor.matmul(psa[:, 0:n], wv8v[:, :, m0:m1],
                                 x8rhs(n0, n, 1), start=False, stop=False,
                                 perf_mode=DR)
                nc.tensor.matmul(psa[:, 0:n], wve8v[:, :, m0:m1],
                                 x8rhs(n0, n), start=False, stop=True,
                                 perf_mode=DR)
            srca = psa[:, 0:n].rearrange("p (r w) -> p r w", w=W)
            nc.scalar.copy(tva3[:, r0:r0 + nr, 1:129], srca)
            psb = pa.tile([128, MM], F32, tag="cv1", name=f"cvb_{ci}")
            nc.tensor.matmul(psb[0:64, 0:n], wv8v[:, :, 128:192],
                             x8rhs(n0, n), start=True, stop=False,
                             perf_mode=DR)
            nc.tensor.matmul(psb[0:64, 0:n], wv8v[:, :, 128:192],
                             x8rhs(n0, n, 1), start=False, stop=False,
                             perf_mode=DR)
            nc.tensor.matmul(psb[0:64, 0:n], wve8v[:, :, 128:192],
                             x8rhs(n0, n), start=False, stop=True,
                             perf_mode=DR)
            srcb = psb[0:64, 0:n].rearrange("p (r w) -> p r w", w=W)
            nc.vector.tensor_copy(tvb3[:, r0:r0 + nr, 1:129], srcb)

        def dw_qk(s):
            dwq = [dwp.tile([128, RS * W], BF16, tag=f"dwq{b}",
                            name=f"dwq{b}_{s}") for b in range(3)]
            for b in range(3):
                dgv = dgqk[:, b * 1280:(b + 1) * 1280]
                for c in range(4):
                    r0 = s * RS + c * 4
                    ps = pa.tile([128, MM], F32, tag="dwps", bufs=2,
                                 name=f"dwps{b}_{s}_{c}")
                    psv = ps[:].rearrange("p (r w) -> p r w", w=W)
                    for pi, (ta, tb) in enumerate(PAIRS):
                        d = TOFF[tb] - TOFF[ta]
                        lhsT = dgv[:, pi * 256:(pi + 1) * 256].rearrange(
                            "p (two m) -> p two m", two=2)
                        rhs = _mk(t_blk[b][:], TOFF[ta] + r0 * WS,
                                  [[d, 2], [WS, 4], [1, W]])
                        nc.tensor.matmul(psv, lhsT, rhs, start=(pi == 0),
                                         stop=(pi == 4), perf_mode=DR)
                    dst = dwq[b][:, c * MM:(c + 1) * MM]
                    if (b + c) % 2 == 0:
                        nc.scalar.copy(dst, ps[:])
                    else:
                        nc.vector.tensor_copy(dst, ps[:])
            return dwq

        def stripe_tr(s, dwq):
            stk = stkp.tile([128, 3 * RS * W], BF16, tag="stk",
                            name=f"stk_{s}", bufs=1)
            for b in range(3):
                dst = stk[:, b * 2048:(b + 1) * 2048].rearrange(
                    "p (n f) -> p n f", f=128)
                nc.sync.dma_start_transpose(dst, dwq[b][:])
            return stk

        def stripe_gram(s, dwq, stk, first, last):
            for i in range(RS):
                st = first and i == 0
                sp = last and i == RS - 1
                lhs0 = stk[:, i * 128:i * 128 + 128]
                lhs1 = stk[:, 2048 + i * 128:2048 + i * 128 + 128]
                rhs3 = _mk(stk[:], i * 128, [[2048, 3], [1, 128]])
                rhs2 = _mk(stk[:], 2048 + i * 128, [[2048, 2], [1, 128]])
                nc.tensor.matmul(g1v, lhs0, rhs3, start=st, stop=sp)
                nc.tensor.matmul(g2v, lhs1, rhs2, start=st, stop=sp)

        def stripe_sq(s, dwq):
            nc.scalar.activation(junk[:], dwq[2][:], ACTF.Square,
                                 accum_out=qn_part[:, s:s + 1])

        def dw_va(s):
            for c in range(4):
                r0 = s * RS + c * 4
                ps = pa.tile([128, MM], F32, tag="cv2", bufs=2,
                             name=f"dwva_{s}_{c}")
                psv = ps[:].rearrange("p (r w) -> p r w", w=W)
                for t in range(9):
                    lhsT = dgva[:, t * 128:(t + 1) * 128]
                    rhs = _mk(tva16[:], TOFF[t] + r0 * WS,
                              [[WS, 4], [1, W]])
                    nc.tensor.matmul(psv, lhsT, rhs, start=(t == 0),
                                     stop=(t == 8))
                dst = v16a[:, r0 * W:(r0 + 4) * W]
                if c % 2 == 0:
                    nc.scalar.copy(dst, ps[:])
                else:
                    nc.vector.tensor_copy(dst, ps[:])

        PE_VB_TAPS = (0, 2, 4, 6)
        DVE_VB_TAPS = (1, 3, 5, 7, 8)

        def dw_vb(s):
            # PE part: 4 taps as bf16 diag matmuls, per 4-row chunk
            for c in range(4):
                r0 = s * RS + c * 4
                ps = pa.tile([128, MM], F32, tag="dwps", bufs=2,
                             name=f"dwvb_{s}_{c}")
                psv = ps[0:64, :].rearrange("p (r w) -> p r w", w=W)
                for ti, t in enumerate(PE_VB_TAPS):
                    lhsT = dgvb16[:, t * 64:(t + 1) * 64]
                    rhs = _mk(tvb16[:], TOFF[t] + r0 * WS,
                              [[WS, 4], [1, W]])
                    nc.tensor.matmul(psv, lhsT, rhs, start=(ti == 0),
                                     stop=(ti == 3))
                dst = vbtmp[:, c * MM:(c + 1) * MM]
                nc.scalar.copy(dst, ps[0:64, :])
            # DVE part: 5 taps + merge with PE partial
            r0 = s * RS
            vb = v16b[:, r0 * W:(r0 + RS) * W]
            vbv = vb.rearrange("p (r w) -> p r w", w=W)
            prod = small.tile([64, RS * W], BF16, tag="vbprod",
                              name=f"vbp_{s}")
            prodv = prod[:].rearrange("p (r w) -> p r w", w=W)
            for ti, t in enumerate(DVE_VB_TAPS):
                dy, dx = TAPS[t]
                view = tvb3[:, r0 + dy:r0 + dy + RS, dx:dx + 128]
                sc = dvb_t[:, t:t + 1]
                if ti == 0:
                    nc.vector.tensor_scalar(vbv, view, sc, None, ALU.mult)
                else:
                    nc.vector.tensor_scalar(prodv, view, sc, None, ALU.mult)
                    nc.vector.tensor_tensor(vb, vb, prod[:], ALU.add)
            nc.vector.tensor_tensor(vb, vb, vbtmp[:], ALU.add)

        emitted = 0
        for ci in range(len(cchunks)):
            conv_qk(ci)
            while emitted < NS and (ci + 1) * 4 >= (emitted * RS + RS + 2):
                s = emitted
                dwq = dw_qk(s)
                stripe_sq(s, dwq)
                stk = stripe_tr(s, dwq)
                stripe_gram(s, dwq, stk, s == 0, s == NS - 1)
                emitted += 1
        assert emitted == NS
        nc.scalar.copy(gab_sb[:], gAB[:])

        # ---- norms + bounce + collective (still inside psum pool) ----
        mk1 = small.tile([128, 128], F32, tag="mk1")
        mk2 = small.tile([128, 128], F32, tag="mk2")
        nc.gpsimd.tensor_tensor(mk1[:], gab_sb[:, 0:128], eye_t[:], ALU.mult)
        nc.gpsimd.tensor_tensor(mk2[:], gab_sb[:, 384:512], eye_t[:],
                                ALU.mult)
        jk2 = small.tile([128, NS], F32, tag="jk2")
        jk3 = small.tile([128, 128], F32, tag="jk3")
        nc.scalar.activation(jk3[:], mk1[:], ACTF.Copy,
                             accum_out=qn_part[:, NS:NS + 1])
        nc.scalar.activation(jk3[:], mk2[:], ACTF.Copy,
                             accum_out=qn_part[:, NS + 1:NS + 2])
        nc.scalar.activation(jk2[:], qn_part[:, 0:NS], ACTF.Copy,
                             accum_out=qn_part[:, NS + 2:NS + 3])
        qred = qn_part[:, NS:NS + 1]          # qn 0:128
        d2 = qn_part[:, NS + 1:NS + 2]        # kn0:64 | qn128:192
        kred = qn_part[:, NS + 2:NS + 3]      # kn 64:192

        bnc_a = small.tile([128, 26], F32, tag="bnca")
        nc.vector.tensor_copy(bnc_a[:, 24:25], qred)
        bounce_in = drm.tile([C, 26], F32)
        bounce_out = drm.tile([2 * C, 26], F32)
        # head gram blocks, rows q0:128 (g1: r1 k0:64 at cols 128:192,
        # r2 k64:192 at cols 256:384 -> col c<64 -> 128+c ; c>=64 -> 192+c)
        for h in range(6):
            r0, r1 = h * CH, min((h + 1) * CH, 128)
            c0, c1 = h * CH, (h + 1) * CH
            if c1 <= 64:
                nc.sync.dma_start(bnc_a[r0:r1, 0:24],
                                  gab_sb[r0:r1, 128 + c0:128 + c1])
            elif c0 >= 64:
                nc.sync.dma_start(bnc_a[r0:r1, 0:24],
                                  gab_sb[r0:r1, 192 + c0:192 + c1])
            else:
                nc.sync.dma_start(bnc_a[r0:r1, 0:64 - c0],
                                  gab_sb[r0:r1, 128 + c0:192])
                nc.sync.dma_start(bnc_a[r0:r1, 64 - c0:24],
                                  gab_sb[r0:r1, 256:192 + c1])
        nc.sync.dma_start(bounce_in[0:128, 0:25], bnc_a[:, 0:25])
        # kn col 25: rows 0:64 <- d2[0:64]; rows 64:192 <- kred
        nc.scalar.dma_start(bounce_in[0:64, 25:26], d2[0:64])
        nc.scalar.dma_start(bounce_in[64:192, 25:26], kred)
        # q-tail norms col 24 rows 128:192 <- d2[64:128]
        nc.scalar.dma_start(bounce_in[128:192, 24:25], d2[64:128])
        # heads 5b,6,7: rows q-tail = g2 rows 64:128; r2 cols = 384:512
        nc.scalar.dma_start(bounce_in[128:144, 0:24], gab_sb[64:80, 568:592])
        nc.scalar.dma_start(bounce_in[144:168, 0:24], gab_sb[80:104, 592:616])
        nc.scalar.dma_start(bounce_in[168:192, 0:24],
                            gab_sb[104:128, 616:640])
        nc.gpsimd.collective_compute(
            "AllGather", ALU.bypass,
            replica_groups=[[0, 1], [2, 3], [4, 5], [6, 7]],
            ins=[bounce_in[:].opt()], outs=[bounce_out[:].opt()])

        # ---- v phase: fills the collective window ---------------------
        emitted_b = 0
        for ci in range(len(cchunks)):
            conv_v(ci)
            while emitted_b < NS and (ci + 1) * 4 >= \
                    (emitted_b * RS + RS + 2):
                s = emitted_b
                dw_va(s)
                dw_vb(s)
                emitted_b += 1
        assert emitted_b == NS

    ines.close()
    outsb = es.enter_context(tc.tile_pool(name="osb", bufs=2))

    # ================= glue + attn-proj ===============================
    with tc.tile_pool(name="pb", bufs=1, space="PSUM") as pb:
        cmp_a = small.tile([128, 26], F32, tag="cmpa")
        cmp_b = small.tile([64, 26], F32, tag="cmpb")
        tmp_a = small.tile([128, 26], F32, tag="tmpa")
        tmp_b = small.tile([64, 26], F32, tag="tmpb")
        nc.sync.dma_start(cmp_a[:], bounce_out[0:128, :])
        nc.sync.dma_start(tmp_a[:], bounce_out[192:320, :])
        nc.sync.dma_start(cmp_b[:], bounce_out[128:192, :])
        nc.sync.dma_start(tmp_b[:], bounce_out[320:384, :])
        nc.vector.tensor_tensor(cmp_a[:], cmp_a[:], tmp_a[:], ALU.add)
        nc.vector.tensor_tensor(cmp_b[:], cmp_b[:], tmp_b[:], ALU.add)

        kn8 = small.tile([HEADS, CH], F32, tag="kn8")
        kn8t = small.tile([HEADS, CH], F32, tag="kn8t")
        nc.sync.dma_start(
            kn8[:], bounce_out[0:192, :].rearrange(
                "(h c) k -> h c k", c=CH)[:, :, 25])
        nc.sync.dma_start(
            kn8t[:], bounce_out[192:384, :].rearrange(
                "(h c) k -> h c k", c=CH)[:, :, 25])
        nc.vector.tensor_tensor(kn8[:], kn8[:], kn8t[:], ALU.add)

        rq_a = small.tile([128, 3], F32, tag="rqa")
        rq_b = small.tile([64, 3], F32, tag="rqb")
        for ti, (cmp, rq, nrow) in enumerate(((cmp_a, rq_a, 128),
                                              (cmp_b, rq_b, 64))):
            nc.scalar.activation(rq[:, 0:1], cmp[:, 24:25], ACTF.Sqrt)
            nc.vector.reciprocal(rq[:, 1:2], rq[:, 0:1])
            nc.vector.tensor_scalar(rq[:, 2:3], rq[:, 1:2],
                                    tmpq_t[0:nrow, ti:ti + 1], None,
                                    ALU.mult)
        rk8 = small.tile([HEADS, 2 * CH], F32, tag="rk8")
        nc.scalar.activation(rk8[:, 0:CH], kn8[:], ACTF.Sqrt)
        nc.vector.reciprocal(rk8[:, CH:2 * CH], rk8[:, 0:CH])
        rk8b = small.tile([HEADS, CH], BF16, tag="rk8b")
        nc.vector.tensor_copy(rk8b[:], rk8[:, CH:2 * CH])

        knb_a = small.tile([128, CH], F32, tag="knba")
        knb_b = small.tile([64, CH], F32, tag="knbb")
        knb_ps = pb.tile([128, MM], F32, tag="pja", name="knb_ps", bufs=2)
        nc.tensor.matmul(knb_ps[:, 0:CH], em_t[:, 0:128], rk8b[:],
                         start=True, stop=True)
        nc.scalar.copy(knb_a[:], knb_ps[:, 0:CH])
        knb_ps2 = pb.tile([128, MM], F32, tag="pja", name="knb_ps2", bufs=2)
        nc.tensor.matmul(knb_ps2[0:64, 0:CH], em_t[:, 128:192], rk8b[:],
                         start=True, stop=True)
        nc.scalar.copy(knb_b[:], knb_ps2[0:64, 0:CH])

        attn16 = small.tile([128, CH], BF16, tag="att16a")
        attn16b = small.tile([64, CH], BF16, tag="att16b")
        for cmp, rq, knb, a16, nrow in ((cmp_a, rq_a, knb_a, attn16, 128),
                                        (cmp_b, rq_b, knb_b, attn16b, 64)):
            at = small.tile([128, CH], F32, tag="atf")
            sm = small.tile([128, 4], F32, tag="smx")
            nc.vector.tensor_scalar(at[0:nrow, :], cmp[0:nrow, 0:CH],
                                    rq[:, 2:3], None, ALU.mult)
            nc.vector.tensor_tensor(at[0:nrow, :], at[0:nrow, :], knb[:],
                                    ALU.mult)
            nc.vector.tensor_reduce(sm[0:nrow, 0:1], at[0:nrow, :], AX.X,
                                    ALU.max)
            nc.vector.tensor_scalar(at[0:nrow, :], at[0:nrow, :],
                                    sm[0:nrow, 0:1], None, ALU.subtract)
            nc.scalar.activation(at[0:nrow, :], at[0:nrow, :], ACTF.Exp)
            nc.vector.tensor_reduce(sm[0:nrow, 1:2], at[0:nrow, :], AX.X,
                                    ALU.add)
            nc.vector.reciprocal(sm[0:nrow, 2:3], sm[0:nrow, 1:2])
            nc.vector.tensor_scalar(a16[0:nrow, :], at[0:nrow, :],
                                    sm[0:nrow, 2:3], None, ALU.mult)

        # BD via stride-0 head-repeat x mask
        bd_a = small.tile([128, C], BF16, tag="bda")
        bd_b = small.tile([64, C], BF16, tag="bdb")
        rep_a = _mk(attn16[:], 0, [[0, HEADS], [1, CH]])
        rep_b = _mk(attn16b[:], 0, [[0, HEADS], [1, CH]])
        nc.vector.tensor_tensor(
            bd_a[:].rearrange("p (h c) -> p h c", c=CH), rep_a,
            emba[:].rearrange("p (h c) -> p h c", c=CH), ALU.mult)
        nc.vector.tensor_tensor(
            bd_b[:].rearrange("p (h c) -> p h c", c=CH), rep_b,
            embb[:].rearrange("p (h c) -> p h c", c=CH), ALU.mult)

        wpp_a16 = small.tile([128, C], BF16, tag="wppa")
        wpp_b16 = small.tile([64, C], BF16, tag="wppb")
        wpp_ps = pb.tile([128, MM], F32, tag="pjb", name="wpp_ps", bufs=2)
        nc.tensor.matmul(wpp_ps[:, 0:C], bd_a[:, 0:128], wpa[:],
                         start=True, stop=False)
        nc.tensor.matmul(wpp_ps[:, 0:C], bd_b[:, 0:128], wpb[:],
                         start=False, stop=True)
        nc.scalar.copy(wpp_a16[:], wpp_ps[:, 0:C])
        wpp_ps2 = pb.tile([128, MM], F32, tag="pjb", name="wpp_ps2", bufs=2)
        nc.tensor.matmul(wpp_ps2[0:64, 0:C], bd_a[:, 128:192], wpa[:],
                         start=True, stop=False)
        nc.tensor.matmul(wpp_ps2[0:64, 0:C], bd_b[:, 128:192], wpb[:],
                         start=False, stop=True)
        nc.scalar.copy(wpp_b16[:], wpp_ps2[0:64, 0:C])

        for g in range(4):
            oa = outsb.tile([128, 2048], BF16, tag="oa", name=f"oa_{g}")
            ob = outsb.tile([64, 2048], BF16, tag="ob", name=f"ob_{g}")
            for cc in range(4):
                n0 = g * 2048 + cc * MM
                pja = pb.tile([128, MM], F32, tag="pja", bufs=2,
                              name=f"pja_{g}_{cc}")
                pjb = pb.tile([64, MM], F32, tag="pjb", bufs=2,
                              name=f"pjb_{g}_{cc}")
                nc.tensor.matmul(pja[:], wpp_a16[:, 0:128],
                                 v16a[:, n0:n0 + MM], start=True, stop=False)
                nc.tensor.matmul(pja[:], wpp_b16[:, 0:128],
                                 v16b[:, n0:n0 + MM], start=False, stop=True)
                nc.tensor.matmul(pjb[:], wpp_a16[:, 128:192],
                                 v16a[:, n0:n0 + MM], start=True, stop=False)
                nc.tensor.matmul(pjb[:], wpp_b16[:, 128:192],
                                 v16b[:, n0:n0 + MM], start=False, stop=True)
                nc.scalar.copy(oa[:, cc * MM:(cc + 1) * MM], pja[:])
                nc.vector.tensor_copy(ob[:, cc * MM:(cc + 1) * MM], pjb[:])
            nc.scalar.dma_start(io["outp"][0:128, g * 2048:(g + 1) * 2048],
                                oa[:])
            nc.scalar.dma_start(io["outp"][128:192, g * 2048:(g + 1) * 2048],
                                ob[:])


# ======================================================================
def _interleave2(w, cols):
    out = np.zeros((128, 2, cols), np.float32)
    out[:, 0, :] = w[0:128]
    out[0:64, 1, :] = w[128:192]
    return out


def _diag_pair_block(wcols, mw):
    npair = len(wcols) // 2
    out = np.zeros((mw, npair, 2, mw), np.float32)
    idx = np.arange(mw)
    for p in range(npair):
        out[idx, p, 0, idx] = wcols[2 * p]
        out[idx, p, 1, idx] = wcols[2 * p + 1]
    return out.reshape(mw, npair * 2 * mw)


def _prep_inputs(x, y, qkv_w, dw_w, proj_w, temperature):
    f8 = lambda a: a.astype(FP8NP)
    f8v = lambda a: a.astype(FP8NP).astype(np.float32)

    WqT = np.ascontiguousarray(qkv_w[0:C].T)
    WkT = np.ascontiguousarray(qkv_w[C:2 * C].T)
    WvT = np.ascontiguousarray(qkv_w[2 * C:3 * C].T)

    wqk = np.zeros((128, 2, 448), np.float32)
    wqk[:, :, 0:128] = _interleave2(WqT, C)[:, :, 0:128]
    wqk[:, :, 192:256] = _interleave2(WqT, C)[:, :, 128:192]
    wqk[:, :, 256:320] = _interleave2(WkT, C)[:, :, 0:64]
    wqk[:, :, 320:448] = _interleave2(WkT, C)[:, :, 64:192]
    wqk8 = f8(wqk.reshape(128, 2 * 448))

    wv = _interleave2(WvT, C)
    wv8 = f8(wv)
    wve8 = f8(wv - wv8.astype(np.float32))

    dw = dw_w.reshape(3 * C, 9).astype(np.float32)
    dw_q, dw_k, dw_v = dw[0:C], dw[C:2 * C], dw[2 * C:3 * C]
    blocks = [dw_q[0:128],
              np.concatenate([dw_k[0:64], dw_q[128:192]], axis=0),
              dw_k[64:192]]
    dgqk = np.zeros((128, 3, 5 * 256), np.float32)
    for b, blk in enumerate(blocks):
        cols = [blk[:, t] for t in range(9)]
        cols.append(np.zeros(128, np.float32))
        dgqk[:, b, :] = _diag_pair_block(cols, 128)
    dgqk8 = f8(dgqk.reshape(128, 3 * 5 * 256))

    dva = dw_v[0:128].astype(np.float32)
    dgva = np.zeros((128, 9, 128), np.float32)
    idx = np.arange(128)
    for t in range(9):
        dgva[idx, t, idx] = dva[:, t]
    dgva = dgva.reshape(128, 9 * 128).astype(BF16NP)
    dvb = np.ascontiguousarray(dw_v[128:192].astype(np.float32))
    dgvb16 = np.zeros((64, 9, 64), np.float32)
    idx64 = np.arange(64)
    for t in range(9):
        dgvb16[idx64, t, idx64] = dvb[:, t]
    dgvb16 = dgvb16.reshape(64, 9 * 64).astype(BF16NP)

    WpT = np.ascontiguousarray(proj_w.T).astype(np.float32)
    wpa = WpT[0:128].astype(BF16NP)
    wpb = WpT[128:192].astype(BF16NP)
    tmpq_full = np.repeat(np.asarray(temperature, np.float32).reshape(HEADS),
                          CH)
    tmpq = np.zeros((128, 2), np.float32)
    tmpq[:, 0] = tmpq_full[0:128]
    tmpq[0:64, 1] = tmpq_full[128:192]
    em = np.zeros((HEADS, C), np.float32)
    for hh in range(HEADS):
        em[hh, hh * CH:(hh + 1) * CH] = 1.0
    emb = np.zeros((C, C), np.float32)
    for cc in range(C):
        hh = cc // CH
        emb[cc, hh * CH:(hh + 1) * CH] = 1.0

    in_maps = []
    for core in range(8):
        bi, half = core // 2, core % 2
        r0 = half * HOUT - 1
        xsl = np.zeros((C, HIN, W), np.float32)
        ysl = np.zeros((C, HIN, W), np.float32)
        lo, hi = max(r0, 0), min(r0 + HIN, 2 * HOUT)
        xsl[:, lo - r0:hi - r0] = x[bi, :, lo:hi]
        ysl[:, lo - r0:hi - r0] = y[bi, :, lo:hi]
        xf = xsl.reshape(C, PXIN)
        yf = ysl.reshape(C, PXIN)
        x8 = xf.astype(FP8NP).astype(np.float32)
        xe8 = f8(xf - x8)
        x4 = np.zeros((128, 2, 2, PXIN), FP8NP)
        x4[:, 0, 0, :] = f8(x8[0:128])
        x4[0:64, 1, 0, :] = f8(x8[128:192])
        x4[:, 0, 1, :] = xe8[0:128]
        x4[0:64, 1, 1, :] = xe8[128:192]
        y8 = np.zeros((128, 2, PXIN), FP8NP)
        y8[:, 0, :] = f8(yf[0:128])
        y8[0:64, 1, :] = f8(yf[128:192])
        in_maps.append({
            "y8": y8.reshape(128, 2 * PXIN),
            "x4": x4.reshape(128, 4 * PXIN),
            "wqk8": wqk8, "wv8": f8(wv8.reshape(128, 2 * 192)),
            "wve8": wve8.reshape(128, 2 * 192),
            "dgqk": dgqk8, "dgva": dgva, "dvb": dvb, "dgvb16": dgvb16,
            "wpa": wpa, "wpb": wpb, "em": em.astype(BF16NP),
            "emba": emb[0:128].astype(BF16NP),
            "embb": emb[128:192].astype(BF16NP),
            "eye": np.eye(128, dtype=np.float32), "tmpq": tmpq,
        })
    return in_maps


def kernel(x, y, qkv_w, dw_w, proj_w, temperature, _trace=False):
    x = np.asarray(x, np.float32)
    y = np.asarray(y, np.float32)
    if "nc" not in _CACHE:
        _CACHE["nc"] = build_program()
    nc = _CACHE["nc"]
    in_maps = _prep_inputs(x, y, np.asarray(qkv_w, np.float32),
                           np.asarray(dw_w, np.float32),
                           np.asarray(proj_w, np.float32),
                           np.asarray(temperature, np.float32))
    res = bass_utils.run_bass_kernel_spmd(nc, in_maps,
                                          core_ids=list(range(8)),
                                          trace=_trace)
    _CACHE["last_result"] = res
    out = np.empty((4, C, 2 * HOUT, W), np.float32)
    for core in range(8):
        bi, half = core // 2, core % 2
        out[bi, :, half * HOUT:(half + 1) * HOUT] = \
            res.results[core]["outp"].astype(np.float32).reshape(C, HOUT, W)
    return out


# revision 22
# speedup vs baseline: 1.0048x; 1.0048x over previous
"""Trainium2 Bass kernel for nn_Attention (channel attention, XCA-style) v3.

Sharding: 8 cores = (batch b=core//2) x (image half = core%2, 64 rows + halo).
Cross-core: AllGather of tiny gram stats over core pairs + local add.

Fully interleaved single pass (one 8-bank PSUM pool):
  per 512-px chunk: conv q,k (fp8 DoubleRow) + conv v (fp8 DR with
  value/residual/weight-residual passes) -> padded fp8/bf16 t-buffers;
  per 16-row stripe: DW q,k via diag tap-pair DR matmuls -> bf16 stripes ->
  batched DMA-xbar transpose -> gram matmuls + Sc Square norms; DW v:
  128-chan block via DR with (value,residual) pair + tap-paired
  weight-error correction, 64-chan block on DVE (bf16 exact).
  AllGather([192,26]) pairs -> local add -> softmax glue ->
  fused (Wp @ blockdiag(attn)) @ v -> bf16 out.
"""

import sys
import numpy as np

sys.path.insert(0, "/opt/trn_rl_repo")

import contextlib  # noqa: E402

import ml_dtypes  # noqa: E402

from concourse import bass, bacc, tile, mybir  # noqa: E402
from concourse import bass_utils  # noqa: E402
from concourse.ap import AP  # noqa: E402

F32 = mybir.dt.float32
BF16 = mybir.dt.bfloat16
FP8 = mybir.dt.float8e4
ALU = mybir.AluOpType
ACTF = mybir.ActivationFunctionType
AX = mybir.AxisListType
BF16NP = ml_dtypes.bfloat16
FP8NP = ml_dtypes.float8_e4m3
DR = mybir.MatmulPerfMode.DoubleRow

C = 192
HEADS = 8
CH = 24
W = 128
HOUT = 64
HIN = HOUT + 2
PXIN = HIN * W            # 8448
PXOUT = HOUT * W          # 8192
WS = 130                  # padded row stride in t buffers
LT = HIN * WS             # 8580
RS = 16                   # stripe out-rows
NS = HOUT // RS           # 4 stripes
MM = 512

TAPS = [(dy, dx) for dy in (0, 1, 2) for dx in (0, 1, 2)]
TOFF = [dy * WS + dx for dy, dx in TAPS]
PAIRS = [(0, 1), (2, 3), (4, 5), (6, 7), (8, 8)]

_CACHE = {}


def _chunks(total, step):
    out, s = [], 0
    while s < total:
        out.append((s, min(step, total - s)))
        s += step
    return out


def _mk(base_ap, off, dims):
    ap0 = [list(base_ap.ap[0])]
    return AP(base_ap.tensor, base_ap.offset + off,
              ap0 + [list(d) for d in dims])


def build_program():
    nc = bacc.Bacc("TRN2", target_bir_lowering=False, debug=False,
                   enable_asserts=False, num_devices=8)
    io = {}
    io["y8"] = nc.dram_tensor("y8", [128, 2 * PXIN], FP8,
                              kind="ExternalInput").ap()
    io["x4"] = nc.dram_tensor("x4", [128, 4 * PXIN], FP8,
                              kind="ExternalInput").ap()
    io["wqk8"] = nc.dram_tensor("wqk8", [128, 2 * 448], FP8,
                                kind="ExternalInput").ap()
    io["wv8"] = nc.dram_tensor("wv8", [128, 2 * 192], FP8,
                               kind="ExternalInput").ap()
    io["wve8"] = nc.dram_tensor("wve8", [128, 2 * 192], FP8,
                                kind="ExternalInput").ap()
    io["dgqk"] = nc.dram_tensor("dgqk", [128, 3 * 5 * 256], FP8,
                                kind="ExternalInput").ap()
    io["dgva"] = nc.dram_tensor("dgva", [128, 9 * 128], BF16,
                                kind="ExternalInput").ap()
    io["dvb"] = nc.dram_tensor("dvb", [64, 9], F32,
                               kind="ExternalInput").ap()
    io["dgvb16"] = nc.dram_tensor("dgvb16", [64, 9 * 64], BF16,
                                  kind="ExternalInput").ap()
    io["wpa"] = nc.dram_tensor("wpa", [128, C], BF16,
                               kind="ExternalInput").ap()
    io["wpb"] = nc.dram_tensor("wpb", [64, C], BF16,
                               kind="ExternalInput").ap()
    io["em"] = nc.dram_tensor("em", [HEADS, C], BF16,
                              kind="ExternalInput").ap()
    io["emba"] = nc.dram_tensor("emba", [128, C], BF16,
                                kind="ExternalInput").ap()
    io["embb"] = nc.dram_tensor("embb", [64, C], BF16,
                                kind="ExternalInput").ap()
    io["eye"] = nc.dram_tensor("eye", [128, 128], F32,
                               kind="ExternalInput").ap()
    io["tmpq"] = nc.dram_tensor("tmpq", [128, 2], F32,
                                kind="ExternalInput").ap()
    io["outp"] = nc.dram_tensor("outp", [C, PXOUT], BF16,
                                kind="ExternalOutput").ap()

    with tile.TileContext(nc) as tc, contextlib.ExitStack() as es:
        _emit(nc, tc, io, es)
    nc.compile()
    return nc


def _emit(nc, tc, io, es):
    # ---------------- pools & persistent tiles ------------------------
    wpool = es.enter_context(tc.tile_pool(name="w", bufs=1))
    dgqk = wpool.tile([128, 3 * 5 * 256], FP8, tag="dgqk")
    dgva = wpool.tile([128, 9 * 128], BF16, tag="dgva")
    dvb_t = wpool.tile([64, 9], F32, tag="dvb")
    dgvb16 = wpool.tile([64, 9 * 64], BF16, tag="dgvb16")
    wpa = wpool.tile([128, C], BF16, tag="wpa")
    wpb = wpool.tile([64, C], BF16, tag="wpb")
    em_t = wpool.tile([HEADS, C], BF16, tag="em")
    emba = wpool.tile([128, C], BF16, tag="emba")
    embb = wpool.tile([64, C], BF16, tag="embb")
    eye_t = wpool.tile([128, 128], F32, tag="eye")
    tmpq_t = wpool.tile([128, 2], F32, tag="tmpq")

    tpool = es.enter_context(tc.tile_pool(name="t", bufs=1))
    t_blk = [tpool.tile([128, LT], FP8, tag=f"t{b}", name=f"t{b}")
             for b in range(3)]
    tva16 = tpool.tile([128, LT], BF16, tag="tva16")
    tvb16 = tpool.tile([64, LT], BF16, tag="tvb16")

    dwp = es.enter_context(tc.tile_pool(name="dw", bufs=2))
    stkp = es.enter_context(tc.tile_pool(name="stk", bufs=1))
    vp = es.enter_context(tc.tile_pool(name="v", bufs=1))
    v16a = vp.tile([128, PXOUT], BF16, tag="v16a")
    v16b = vp.tile([64, PXOUT], BF16, tag="v16b")
    vbtmp = vp.tile([64, RS * W], BF16, tag="vbtmp")
    small = es.enter_context(tc.tile_pool(name="sm", bufs=1))
    drm = es.enter_context(tc.tile_pool(name="drm", bufs=1, space="DRAM"))

    ines = contextlib.ExitStack()
    inpool = ines.enter_context(tc.tile_pool(name="inp", bufs=1))
    y8 = inpool.tile([128, 2 * PXIN], FP8, tag="y8")
    x4 = inpool.tile([128, 4 * PXIN], FP8, tag="x4")
    wqk8 = inpool.tile([128, 2 * 448], FP8, tag="wqk8")
    wv8 = inpool.tile([128, 2 * 192], FP8, tag="wv8")
    wve8 = inpool.tile([128, 2 * 192], FP8, tag="wve8")

    # conv weights first on the sync queue (gate the first matmuls)
    for nm, t in (("wqk8", wqk8), ("wv8", wv8), ("wve8", wve8),
                  ("dgqk", dgqk), ("dgva", dgva), ("dvb", dvb_t),
                  ("dgvb16", dgvb16)):
        nc.sync.dma_start(t[:], io[nm])
    # inputs: split by pixel range, both channel-halves per range (Pool q)
    for part in range(4):
        a, b = part * PXIN // 4, (part + 1) * PXIN // 4
        nc.gpsimd.dma_start(y8[:, a:b], io["y8"][:, a:b])
        nc.gpsimd.dma_start(y8[:, PXIN + a:PXIN + b],
                            io["y8"][:, PXIN + a:PXIN + b])
        for sec in range(4):
            o = sec * PXIN
            nc.gpsimd.dma_start(x4[:, o + a:o + b], io["x4"][:, o + a:o + b])
    for nm, t in (("wpa", wpa), ("wpb", wpb), ("em", em_t), ("eye", eye_t),
                  ("emba", emba), ("embb", embb), ("tmpq", tmpq_t)):
        nc.sync.dma_start(t[:], io[nm])

    # pad-column zeroing
    for t in t_blk:
        v = t[:].rearrange("p (r w) -> p r w", w=WS)
        nc.vector.memset(v[:, :, 0:1], 0.0)
        nc.vector.memset(v[:, :, 129:130], 0.0)
    v = tva16[:].rearrange("p (r w) -> p r w", w=WS)
    nc.vector.memset(v[:, :, 0:1], 0.0)
    nc.vector.memset(v[:, :, 129:130], 0.0)
    v = tvb16[:].rearrange("p (r w) -> p r w", w=WS)
    nc.vector.memset(v[:, :, 0:1], 0.0)
    nc.vector.memset(v[:, :, 129:130], 0.0)

    w2v = wqk8[:].rearrange("p (two m) -> p two m", two=2)
    wv8v = wv8[:].rearrange("p (two m) -> p two m", two=2)
    wve8v = wve8[:].rearrange("p (two m) -> p two m", two=2)

    def y8rhs(n0, n):
        return _mk(y8[:], n0, [[PXIN, 2], [1, n]])

    def x8rhs(n0, n, res=0):
        return _mk(x4[:], res * PXIN + n0, [[2 * PXIN, 2], [1, n]])

    cchunks = _chunks(PXIN, MM)
    gab_sb = small.tile([128, 640], F32, tag="gabsb")
    qn_part = small.tile([128, 3 * NS + 4], F32, tag="qnp")
    junk = small.tile([128, RS * W], BF16, tag="junk")
    tva3 = tva16[:].rearrange("p (r w) -> p r w", w=WS)
    tvb3 = tvb16[:].rearrange("p (r w) -> p r w", w=WS)

    # ============ fused pass: conv qk+v, DW, gram =====================
    with tc.tile_pool(name="pa", bufs=1, space="PSUM") as pa:
        gAB = pa.tile([128, 640], F32, tag="gAB")
        g1v = gAB[:, 0:384].rearrange("p (r c) -> p r c", c=128)
        g2v = gAB[:, 384:640].rearrange("p (r c) -> p r c", c=128)

        def conv_qk(ci):
            n0, n = cchunks[ci]
            r0, nr = n0 // W, n // W
            ps0 = pa.tile([128, MM], F32, tag="cv0", name=f"cv0_{ci}")
            ps1 = pa.tile([128, MM], F32, tag="cv1", name=f"cv1_{ci}")
            ps2 = pa.tile([128, MM], F32, tag="cv2", name=f"cv2_{ci}",
                          bufs=2)
            nc.tensor.matmul(ps0[:, 0:n], w2v[:, :, 0:128], y8rhs(n0, n),
                             start=True, stop=True, perf_mode=DR)
            # t1 = [k0:64 @ parts 0:64 ; q128:192 @ parts 64:128]
            nc.tensor.matmul(ps1[:, 0:n], w2v[:, :, 128:256], y8rhs(n0, n),
                             start=True, stop=False, perf_mode=DR)
            nc.tensor.matmul(ps1[0:64, 0:n], w2v[:, :, 256:320],
                             x8rhs(n0, n), start=False, stop=True,
                             perf_mode=DR, skip_group_check=True)
            nc.tensor.matmul(ps2[:, 0:n], w2v[:, :, 320:448], x8rhs(n0, n),
                             start=True, stop=True, perf_mode=DR)
            for b, ps, eng in ((0, ps0, 0), (1, ps1, 1), (2, ps2, 1)):
                dst = t_blk[b][:].rearrange("p (r w) -> p r w", w=WS)
                src = ps[:, 0:n].rearrange("p (r w) -> p r w", w=W)
                if eng == 0:
                    nc.scalar.copy(dst[:, r0:r0 + nr, 1:129], src)
                else:
                    nc.vector.tensor_copy(dst[:, r0:r0 + nr, 1:129], src)

        def conv_v(ci):
            n0, n = cchunks[ci]
            r0, nr = n0 // W, n // W
            psa = pa.tile([128, MM], F32, tag="cv0", name=f"cva_{ci}")
            for m0, m1 in ((0, 128),):
                nc.tensor.matmul(psa[:, 0:n], wv8v[:, :, m0:m1],
                                 x8rhs(n0, n), start=True, stop=False,
                                 perf_mode=DR)
                nc.tensor.matmul(psa[:, 0:n], wv8v[:, :, m0:m1],
                                 x8rhs(n0, n, 1), start=False, stop=False,
                                 perf_mode=DR)
                nc.tensor.matmul(psa[:, 0:n], wve8v[:, :, m0:m1],
                                 x8rhs(n0, n), start=False, stop=True,
                                 perf_mode=DR)
            srca = psa[:, 0:n].rearrange("p (r w) -> p r w", w=W)
            nc.scalar.copy(tva3[:, r0:r0 + nr, 1:129], srca)
            psb = pa.tile([128, MM], F32, tag="cv1", name=f"cvb_{ci}")
            nc.tensor.matmul(psb[0:64, 0:n], wv8v[:, :, 128:192],
                             x8rhs(n0, n), start=True, stop=False,
                             perf_mode=DR)
            nc.tensor.matmul(psb[0:64, 0:n], wv8v[:, :, 128:192],
                             x8rhs(n0, n, 1), start=False, stop=False,
                             perf_mode=DR)
            nc.tensor.matmul(psb[0:64, 0:n], wve8v[:, :, 128:192],
                             x8rhs(n0, n), start=False, stop=True,
                             perf_mode=DR)
            srcb = psb[0:64, 0:n].rearrange("p (r w) -> p r w", w=W)
            nc.vector.tensor_copy(tvb3[:, r0:r0 + nr, 1:129], srcb)

        def dw_qk(s):
            dwq = [dwp.tile([128, RS * W], BF16, tag=f"dwq{b}",
                            name=f"dwq{b}_{s}") for b in range(3)]
            for b in range(3):
                dgv = dgqk[:, b * 1280:(b + 1) * 1280]
                for c in range(4):
                    r0 = s * RS + c * 4
                    ps = pa.tile([128, MM], F32, tag="dwps", bufs=2,
                                 name=f"dwps{b}_{s}_{c}")
                    psv = ps[:].rearrange("p (r w) -> p r w", w=W)
                    for pi, (ta, tb) in enumerate(PAIRS):
                        d = TOFF[tb] - TOFF[ta]
                        lhsT = dgv[:, pi * 256:(pi + 1) * 256].rearrange(
                            "p (two m) -> p two m", two=2)
                        rhs = _mk(t_blk[b][:], TOFF[ta] + r0 * WS,
                                  [[d, 2], [WS, 4], [1, W]])
                        nc.tensor.matmul(psv, lhsT, rhs, start=(pi == 0),
                                         stop=(pi == 4), perf_mode=DR)
                    dst = dwq[b][:, c * MM:(c + 1) * MM]
                    if (b + c) % 2 == 0:
                        nc.scalar.copy(dst, ps[:])
                    else:
                        nc.vector.tensor_copy(dst, ps[:])
            return dwq

        def stripe_tr(s, dwq):
            stk = stkp.tile([128, 3 * RS * W], BF16, tag="stk",
                            name=f"stk_{s}", bufs=1)
            for b in range(3):
                dst = stk[:, b * 2048:(b + 1) * 2048].rearrange(
                    "p (n f) -> p n f", f=128)
                nc.sync.dma_start_transpose(dst, dwq[b][:])
            return stk

        def stripe_gram(s, dwq, stk, first, last):
            for i in range(RS):
                st = first and i == 0
                sp = last and i == RS - 1
                lhs0 = stk[:, i * 128:i * 128 + 128]
                lhs1 = stk[:, 2048 + i * 128:2048 + i * 128 + 128]
                rhs3 = _mk(stk[:], i * 128, [[2048, 3], [1, 128]])
                rhs2 = _mk(stk[:], 2048 + i * 128, [[2048, 2], [1, 128]])
                nc.tensor.matmul(g1v, lhs0, rhs3, start=st, stop=sp)
                nc.tensor.matmul(g2v, lhs1, rhs2, start=st, stop=sp)

        def stripe_sq(s, dwq):
            nc.scalar.activation(junk[:], dwq[2][:], ACTF.Square,
                                 accum_out=qn_part[:, s:s + 1])

        def dw_va(s):
            for c in range(4):
                r0 = s * RS + c * 4
                ps = pa.tile([128, MM], F32, tag="cv2", bufs=2,
                             name=f"dwva_{s}_{c}")
                psv = ps[:].rearrange("p (r w) -> p r w", w=W)
                for t in range(9):
                    lhsT = dgva[:, t * 128:(t + 1) * 128]
                    rhs = _mk(tva16[:], TOFF[t] + r0 * WS,
                              [[WS, 4], [1, W]])
                    nc.tensor.matmul(psv, lhsT, rhs, start=(t == 0),
                                     stop=(t == 8))
                dst = v16a[:, r0 * W:(r0 + 4) * W]
                if c % 2 == 0:
                    nc.scalar.copy(dst, ps[:])
                else:
                    nc.vector.tensor_copy(dst, ps[:])

        PE_VB_TAPS = (0, 2, 4, 6)
        DVE_VB_TAPS = (1, 3, 5, 7, 8)

        def dw_vb(s):
            # PE part: 4 taps as bf16 diag matmuls, per 4-row chunk
            for c in range(4):
                r0 = s * RS + c * 4
                ps = pa.tile([128, MM], F32, tag="dwps", bufs=2,
                             name=f"dwvb_{s}_{c}")
                psv = ps[0:64, :].rearrange("p (r w) -> p r w", w=W)
                for ti, t in enumerate(PE_VB_TAPS):
                    lhsT = dgvb16[:, t * 64:(t + 1) * 64]
                    rhs = _mk(tvb16[:], TOFF[t] + r0 * WS,
                              [[WS, 4], [1, W]])
                    nc.tensor.matmul(psv, lhsT, rhs, start=(ti == 0),
                                     stop=(ti == 3))
                dst = vbtmp[:, c * MM:(c + 1) * MM]
                nc.scalar.copy(dst, ps[0:64, :])
            # DVE part: 5 taps + merge with PE partial
            r0 = s * RS
            vb = v16b[:, r0 * W:(r0 + RS) * W]
            vbv = vb.rearrange("p (r w) -> p r w", w=W)
            prod = small.tile([64, RS * W], BF16, tag="vbprod",
                              name=f"vbp_{s}")
            prodv = prod[:].rearrange("p (r w) -> p r w", w=W)
            for ti, t in enumerate(DVE_VB_TAPS):
                dy, dx = TAPS[t]
                view = tvb3[:, r0 + dy:r0 + dy + RS, dx:dx + 128]
                sc = dvb_t[:, t:t + 1]
                if ti == 0:
                    nc.vector.tensor_scalar(vbv, view, sc, None, ALU.mult)
                else:
                    nc.vector.tensor_scalar(prodv, view, sc, None, ALU.mult)
                    nc.vector.tensor_tensor(vb, vb, prod[:], ALU.add)
            nc.vector.tensor_tensor(vb, vb, vbtmp[:], ALU.add)

        emitted = 0
        for ci in range(len(cchunks)):
            conv_qk(ci)
            while emitted < NS and (ci + 1) * 4 >= (emitted * RS + RS + 2):
                s = emitted
                dwq = dw_qk(s)
                stripe_sq(s, dwq)
                stk = stripe_tr(s, dwq)
                stripe_gram(s, dwq, stk, s == 0, s == NS - 1)
                emitted += 1
        assert emitted == NS
        nc.scalar.copy(gab_sb[:], gAB[:])

        # ---- norms + bounce + collective (still inside psum pool) ----
        mk1 = small.tile([128, 128], F32, tag="mk1")
        mk2 = small.tile([128, 128], F32, tag="mk2")
        nc.gpsimd.tensor_tensor(mk1[:], gab_sb[:, 0:128], eye_t[:], ALU.mult)
        nc.gpsimd.tensor_tensor(mk2[:], gab_sb[:, 384:512], eye_t[:],
                                ALU.mult)
        jk2 = small.tile([128, NS], F32, tag="jk2")
        jk3 = small.tile([128, 128], F32, tag="jk3")
        nc.scalar.activation(jk3[:], mk1[:], ACTF.Copy,
                             accum_out=qn_part[:, NS:NS + 1])
        nc.scalar.activation(jk3[:], mk2[:], ACTF.Copy,
                             accum_out=qn_part[:, NS + 1:NS + 2])
        nc.scalar.activation(jk2[:], qn_part[:, 0:NS], ACTF.Copy,
                             accum_out=qn_part[:, NS + 2:NS + 3])
        qred = qn_part[:, NS:NS + 1]          # qn 0:128
        d2 = qn_part[:, NS + 1:NS + 2]        # kn0:64 | qn128:192
        kred = qn_part[:, NS + 2:NS + 3]      # kn 64:192

        bnc_a = small.tile([128, 26], F32, tag="bnca")
        nc.vector.tensor_copy(bnc_a[:, 24:25], qred)
        bounce_in = drm.tile([C, 26], F32)
        bounce_out = drm.tile([2 * C, 26], F32)
        # head gram blocks, rows q0:128 (g1: r1 k0:64 at cols 128:192,
        # r2 k64:192 at cols 256:384 -> col c<64 -> 128+c ; c>=64 -> 192+c)
        for h in range(6):
            r0, r1 = h * CH, min((h + 1) * CH, 128)
            c0, c1 = h * CH, (h + 1) * CH
            if c1 <= 64:
                nc.sync.dma_start(bnc_a[r0:r1, 0:24],
                                  gab_sb[r0:r1, 128 + c0:128 + c1])
            elif c0 >= 64:
                nc.sync.dma_start(bnc_a[r0:r1, 0:24],
                                  gab_sb[r0:r1, 192 + c0:192 + c1])
            else:
                nc.sync.dma_start(bnc_a[r0:r1, 0:64 - c0],
                                  gab_sb[r0:r1, 128 + c0:192])
                nc.sync.dma_start(bnc_a[r0:r1, 64 - c0:24],
                                  gab_sb[r0:r1, 256:192 + c1])
        nc.sync.dma_start(bounce_in[0:128, 0:25], bnc_a[:, 0:25])
        # kn col 25: rows 0:64 <- d2[0:64]; rows 64:192 <- kred
        nc.scalar.dma_start(bounce_in[0:64, 25:26], d2[0:64])
        nc.scalar.dma_start(bounce_in[64:192, 25:26], kred)
        # q-tail norms col 24 rows 128:192 <- d2[64:128]
        nc.scalar.dma_start(bounce_in[128:192, 24:25], d2[64:128])
        # heads 5b,6,7: rows q-tail = g2 rows 64:128; r2 cols = 384:512
        nc.scalar.dma_start(bounce_in[128:144, 0:24], gab_sb[64:80, 568:592])
        nc.scalar.dma_start(bounce_in[144:168, 0:24], gab_sb[80:104, 592:616])
        nc.scalar.dma_start(bounce_in[168:192, 0:24],
                            gab_sb[104:128, 616:640])
        nc.gpsimd.collective_compute(
            "AllGather", ALU.bypass,
            replica_groups=[[0, 1], [2, 3], [4, 5], [6, 7]],
            ins=[bounce_in[:].opt()], outs=[bounce_out[:].opt()])

        # ---- v phase: fills the collective window ---------------------
        emitted_b = 0
        for ci in range(len(cchunks)):
            conv_v(ci)
            while emitted_b < NS and (ci + 1) * 4 >= \
                    (emitted_b * RS + RS + 2):
                s = emitted_b
                dw_va(s)
                dw_vb(s)
                emitted_b += 1
        assert emitted_b == NS

    ines.close()
    outsb = es.enter_context(tc.tile_pool(name="osb", bufs=2))

    # ================= glue + attn-proj ===============================
    with tc.tile_pool(name="pb", bufs=1, space="PSUM") as pb:
        cmp_a = small.tile([128, 26], F32, tag="cmpa")
        cmp_b = small.tile([64, 26], F32, tag="cmpb")
        tmp_a = small.tile([128, 26], F32, tag="tmpa")
        tmp_b = small.tile([64, 26], F32, tag="tmpb")
        nc.sync.dma_start(cmp_a[:], bounce_out[0:128, :])
        nc.sync.dma_start(tmp_a[:], bounce_out[192:320, :])
        nc.sync.dma_start(cmp_b[:], bounce_out[128:192, :])
        nc.sync.dma_start(tmp_b[:], bounce_out[320:384, :])
        nc.vector.tensor_tensor(cmp_a[:], cmp_a[:], tmp_a[:], ALU.add)
        nc.vector.tensor_tensor(cmp_b[:], cmp_b[:], tmp_b[:], ALU.add)

        kn8 = small.tile([HEADS, CH], F32, tag="kn8")
        kn8t = small.tile([HEADS, CH], F32, tag="kn8t")
        nc.sync.dma_start(
            kn8[:], bounce_out[0:192, :].rearrange(
                "(h c) k -> h c k", c=CH)[:, :, 25])
        nc.sync.dma_start(
            kn8t[:], bounce_out[192:384, :].rearrange(
                "(h c) k -> h c k", c=CH)[:, :, 25])
        nc.vector.tensor_tensor(kn8[:], kn8[:], kn8t[:], ALU.add)

        rq_a = small.tile([128, 3], F32, tag="rqa")
        rq_b = small.tile([64, 3], F32, tag="rqb")
        for ti, (cmp, rq, nrow) in enumerate(((cmp_a, rq_a, 128),
                                              (cmp_b, rq_b, 64))):
            nc.scalar.activation(rq[:, 0:1], cmp[:, 24:25], ACTF.Sqrt)
            nc.vector.reciprocal(rq[:, 1:2], rq[:, 0:1])
            nc.vector.tensor_scalar(rq[:, 2:3], rq[:, 1:2],
                                    tmpq_t[0:nrow, ti:ti + 1], None,
                                    ALU.mult)
        rk8 = small.tile([HEADS, 2 * CH], F32, tag="rk8")
        nc.scalar.activation(rk8[:, 0:CH], kn8[:], ACTF.Sqrt)
        nc.vector.reciprocal(rk8[:, CH:2 * CH], rk8[:, 0:CH])
        rk8b = small.tile([HEADS, CH], BF16, tag="rk8b")
        nc.vector.tensor_copy(rk8b[:], rk8[:, CH:2 * CH])

        knb_a = small.tile([128, CH], F32, tag="knba")
        knb_b = small.tile([64, CH], F32, tag="knbb")
        knb_ps = pb.tile([128, MM], F32, tag="pja", name="knb_ps", bufs=2)
        nc.tensor.matmul(knb_ps[:, 0:CH], em_t[:, 0:128], rk8b[:],
                         start=True, stop=True)
        nc.scalar.copy(knb_a[:], knb_ps[:, 0:CH])
        knb_ps2 = pb.tile([128, MM], F32, tag="pja", name="knb_ps2", bufs=2)
        nc.tensor.matmul(knb_ps2[0:64, 0:CH], em_t[:, 128:192], rk8b[:],
                         start=True, stop=True)
        nc.scalar.copy(knb_b[:], knb_ps2[0:64, 0:CH])

        attn16 = small.tile([128, CH], BF16, tag="att16a")
        attn16b = small.tile([64, CH], BF16, tag="att16b")
        for cmp, rq, knb, a16, nrow in ((cmp_a, rq_a, knb_a, attn16, 128),
                                        (cmp_b, rq_b, knb_b, attn16b, 64)):
            at = small.tile([128, CH], F32, tag="atf")
            sm = small.tile([128, 4], F32, tag="smx")
            nc.vector.tensor_scalar(at[0:nrow, :], cmp[0:nrow, 0:CH],
                                    rq[:, 2:3], None, ALU.mult)
            nc.vector.tensor_tensor(at[0:nrow, :], at[0:nrow, :], knb[:],
                                    ALU.mult)
            nc.vector.tensor_reduce(sm[0:nrow, 0:1], at[0:nrow, :], AX.X,
                                    ALU.max)
            nc.vector.tensor_scalar(at[0:nrow, :], at[0:nrow, :],
                                    sm[0:nrow, 0:1], None, ALU.subtract)
            nc.scalar.activation(at[0:nrow, :], at[0:nrow, :], ACTF.Exp)
            nc.vector.tensor_reduce(sm[0:nrow, 1:2], at[0:nrow, :], AX.X,
                                    ALU.add)
            nc.vector.reciprocal(sm[0:nrow, 2:3], sm[0:nrow, 1:2])
            nc.vector.tensor_scalar(a16[0:nrow, :], at[0:nrow, :],
                                    sm[0:nrow, 2:3], None, ALU.mult)

        # BD via stride-0 head-repeat x mask
        bd_a = small.tile([128, C], BF16, tag="bda")
        bd_b = small.tile([64, C], BF16, tag="bdb")
        rep_a = _mk(attn16[:], 0, [[0, HEADS], [1, CH]])
        rep_b = _mk(attn16b[:], 0, [[0, HEADS], [1, CH]])
        nc.vector.tensor_tensor(
            bd_a[:].rearrange("p (h c) -> p h c", c=CH), rep_a,
            emba[:].rearrange("p (h c) -> p h c", c=CH), ALU.mult)
        nc.vector.tensor_tensor(
            bd_b[:].rearrange("p (h c) -> p h c", c=CH), rep_b,
            embb[:].rearrange("p (h c) -> p h c", c=CH), ALU.mult)

        wpp_a16 = small.tile([128, C], BF16, tag="wppa")
        wpp_b16 = small.tile([64, C], BF16, tag="wppb")
        wpp_ps = pb.tile([128, MM], F32, tag="pjb", name="wpp_ps", bufs=2)
        nc.tensor.matmul(wpp_ps[:, 0:C], bd_a[:, 0:128], wpa[:],
                         start=True, stop=False)
        nc.tensor.matmul(wpp_ps[:, 0:C], bd_b[:, 0:128], wpb[:],
                         start=False, stop=True)
        nc.scalar.copy(wpp_a16[:], wpp_ps[:, 0:C])
        wpp_ps2 = pb.tile([128, MM], F32, tag="pjb", name="wpp_ps2", bufs=2)
        nc.tensor.matmul(wpp_ps2[0:64, 0:C], bd_a[:, 128:192], wpa[:],
                         start=True, stop=False)
        nc.tensor.matmul(wpp_ps2[0:64, 0:C], bd_b[:, 128:192], wpb[:],
                         start=False, stop=True)
        nc.scalar.copy(wpp_b16[:], wpp_ps2[0:64, 0:C])

        for g in range(4):
            oa = outsb.tile([128, 2048], BF16, tag="oa", name=f"oa_{g}")
            ob = outsb.tile([64, 2048], BF16, tag="ob", name=f"ob_{g}")
            for cc in range(4):
                n0 = g * 2048 + cc * MM
                pja = pb.tile([128, MM], F32, tag="pja", bufs=2,
                              name=f"pja_{g}_{cc}")
                pjb = pb.tile([64, MM], F32, tag="pjb", bufs=2,
                              name=f"pjb_{g}_{cc}")
                nc.tensor.matmul(pja[:], wpp_a16[:, 0:128],
                                 v16a[:, n0:n0 + MM], start=True, stop=False)
                nc.tensor.matmul(pja[:], wpp_b16[:, 0:128],
                                 v16b[:, n0:n0 + MM], start=False, stop=True)
                nc.tensor.matmul(pjb[:], wpp_a16[:, 128:192],
                                 v16a[:, n0:n0 + MM], start=True, stop=False)
                nc.tensor.matmul(pjb[:], wpp_b16[:, 128:192],
                                 v16b[:, n0:n0 + MM], start=False, stop=True)
                nc.scalar.copy(oa[:, cc * MM:(cc + 1) * MM], pja[:])
                nc.vector.tensor_copy(ob[:, cc * MM:(cc + 1) * MM], pjb[:])
            nc.scalar.dma_start(io["outp"][0:128, g * 2048:(g + 1) * 2048],
                                oa[:])
            nc.scalar.dma_start(io["outp"][128:192, g * 2048:(g + 1) * 2048],
                                ob[:])


# ======================================================================
def _interleave2(w, cols):
    out = np.zeros((128, 2, cols), np.float32)
    out[:, 0, :] = w[0:128]
    out[0:64, 1, :] = w[128:192]
    return out


def _diag_pair_block(wcols, mw):
    npair = len(wcols) // 2
    out = np.zeros((mw, npair, 2, mw), np.float32)
    idx = np.arange(mw)
    for p in range(npair):
        out[idx, p, 0, idx] = wcols[2 * p]
        out[idx, p, 1, idx] = wcols[2 * p + 1]
    return out.reshape(mw, npair * 2 * mw)


def _prep_inputs(x, y, qkv_w, dw_w, proj_w, temperature):
    f8 = lambda a: a.astype(FP8NP)
    f8v = lambda a: a.astype(FP8NP).astype(np.float32)

    WqT = np.ascontiguousarray(qkv_w[0:C].T)
    WkT = np.ascontiguousarray(qkv_w[C:2 * C].T)
    WvT = np.ascontiguousarray(qkv_w[2 * C:3 * C].T)

    wqk = np.zeros((128, 2, 448), np.float32)
    wqk[:, :, 0:128] = _interleave2(WqT, C)[:, :, 0:128]
    wqk[:, :, 192:256] = _interleave2(WqT, C)[:, :, 128:192]
    wqk[:, :, 256:320] = _interleave2(WkT, C)[:, :, 0:64]
    wqk[:, :, 320:448] = _interleave2(WkT, C)[:, :, 64:192]
    wqk8 = f8(wqk.reshape(128, 2 * 448))

    wv = _interleave2(WvT, C)
    wv8 = f8(wv)
    wve8 = f8(wv - wv8.astype(np.float32))

    dw = dw_w.reshape(3 * C, 9).astype(np.float32)
    dw_q, dw_k, dw_v = dw[0:C], dw[C:2 * C], dw[2 * C:3 * C]
    blocks = [dw_q[0:128],
              np.concatenate([dw_k[0:64], dw_q[128:192]], axis=0),
              dw_k[64:192]]
    dgqk = np.zeros((128, 3, 5 * 256), np.float32)
    for b, blk in enumerate(blocks):
        cols = [blk[:, t] for t in range(9)]
        cols.append(np.zeros(128, np.float32))
        dgqk[:, b, :] = _diag_pair_block(cols, 128)
    dgqk8 = f8(dgqk.reshape(128, 3 * 5 * 256))

    dva = dw_v[0:128].astype(np.float32)
    dgva = np.zeros((128, 9, 128), np.float32)
    idx = np.arange(128)
    for t in range(9):
        dgva[idx, t, idx] = dva[:, t]
    dgva = dgva.reshape(128, 9 * 128).astype(BF16NP)
    dvb = np.ascontiguousarray(dw_v[128:192].astype(np.float32))
    dgvb16 = np.zeros((64, 9, 64), np.float32)
    idx64 = np.arange(64)
    for t in range(9):
        dgvb16[idx64, t, idx64] = dvb[:, t]
    dgvb16 = dgvb16.reshape(64, 9 * 64).astype(BF16NP)

    WpT = np.ascontiguousarray(proj_w.T).astype(np.float32)
    wpa = WpT[0:128].astype(BF16NP)
    wpb = WpT[128:192].astype(BF16NP)
    tmpq_full = np.repeat(np.asarray(temperature, np.float32).reshape(HEADS),
                          CH)
    tmpq = np.zeros((128, 2), np.float32)
    tmpq[:, 0] = tmpq_full[0:128]
    tmpq[0:64, 1] = tmpq_full[128:192]
    em = np.zeros((HEADS, C), np.float32)
    for hh in range(HEADS):
        em[hh, hh * CH:(hh + 1) * CH] = 1.0
    emb = np.zeros((C, C), np.float32)
    for cc in range(C):
        hh = cc // CH
        emb[cc, hh * CH:(hh + 1) * CH] = 1.0

    in_maps = []
    for core in range(8):
        bi, half = core // 2, core % 2
        r0 = half * HOUT - 1
        xsl = np.zeros((C, HIN, W), np.float32)
        ysl = np.zeros((C, HIN, W), np.float32)
        lo, hi = max(r0, 0), min(r0 + HIN, 2 * HOUT)
        xsl[:, lo - r0:hi - r0] = x[bi, :, lo:hi]
        ysl[:, lo - r0:hi - r0] = y[bi, :, lo:hi]
        xf = xsl.reshape(C, PXIN)
        yf = ysl.reshape(C, PXIN)
        x8 = xf.astype(FP8NP).astype(np.float32)
        xe8 = f8(xf - x8)
        x4 = np.zeros((128, 2, 2, PXIN), FP8NP)
        x4[:, 0, 0, :] = f8(x8[0:128])
        x4[0:64, 1, 0, :] = f8(x8[128:192])
        x4[:, 0, 1, :] = xe8[0:128]
        x4[0:64, 1, 1, :] = xe8[128:192]
        y8 = np.zeros((128, 2, PXIN), FP8NP)
        y8[:, 0, :] = f8(yf[0:128])
        y8[0:64, 1, :] = f8(yf[128:192])
        in_maps.append({
            "y8": y8.reshape(128, 2 * PXIN),
            "x4": x4.reshape(128, 4 * PXIN),
            "wqk8": wqk8, "wv8": f8(wv8.reshape(128, 2 * 192)),
            "wve8": wve8.reshape(128, 2 * 192),
            "dgqk": dgqk8, "dgva": dgva, "dvb": dvb, "dgvb16": dgvb16,
            "wpa": wpa, "wpb": wpb, "em": em.astype(BF16NP),
            "emba": emb[0:128].astype(BF16NP),
            "embb": emb[128:192].astype(BF16NP),
            "eye": np.eye(128, dtype=np.float32), "tmpq": tmpq,
        })
    return in_maps


def kernel(x, y, qkv_w, dw_w, proj_w, temperature, _trace=False):
    x = np.asarray(x, np.float32)
    y = np.asarray(y, np.float32)
    if "nc" not in _CACHE:
        _CACHE["nc"] = build_program()
    nc = _CACHE["nc"]
    in_maps = _prep_inputs(x, y, np.asarray(qkv_w, np.float32),
                           np.asarray(dw_w, np.float32),
                           np.asarray(proj_w, np.float32),
                           np.asarray(temperature, np.float32))
    res = bass_utils.run_bass_kernel_spmd(nc, in_maps,
                                          core_ids=list(range(8)),
                                          trace=_trace)
    _CACHE["last_result"] = res
    out = np.empty((4, C, 2 * HOUT, W), np.float32)
    for core in range(8):
        bi, half = core // 2, core % 2
        out[bi, :, half * HOUT:(half + 1) * HOUT] = \
            res.results[core]["outp"].astype(np.float32).reshape(C, HOUT, W)
    return out


# revision 23
# speedup vs baseline: 1.0060x; 1.0012x over previous
"""Trainium2 Bass kernel for nn_Attention (channel attention, XCA-style) v3.

Sharding: 8 cores = (batch b=core//2) x (image half = core%2, 64 rows + halo).
Cross-core: AllGather of tiny gram stats over core pairs + local add.

Fully interleaved single pass (one 8-bank PSUM pool):
  per 512-px chunk: conv q,k (fp8 DoubleRow) + conv v (fp8 DR with
  value/residual/weight-residual passes) -> padded fp8/bf16 t-buffers;
  per 16-row stripe: DW q,k via diag tap-pair DR matmuls -> bf16 stripes ->
  batched DMA-xbar transpose -> gram matmuls + Sc Square norms; DW v:
  128-chan block via DR with (value,residual) pair + tap-paired
  weight-error correction, 64-chan block on DVE (bf16 exact).
  AllGather([192,26]) pairs -> local add -> softmax glue ->
  fused (Wp @ blockdiag(attn)) @ v -> bf16 out.
"""

import sys
import numpy as np

sys.path.insert(0, "/opt/trn_rl_repo")

import contextlib  # noqa: E402

import ml_dtypes  # noqa: E402

from concourse import bass, bacc, tile, mybir  # noqa: E402
from concourse import bass_utils  # noqa: E402
from concourse.ap import AP  # noqa: E402

F32 = mybir.dt.float32
BF16 = mybir.dt.bfloat16
FP8 = mybir.dt.float8e4
ALU = mybir.AluOpType
ACTF = mybir.ActivationFunctionType
AX = mybir.AxisListType
BF16NP = ml_dtypes.bfloat16
FP8NP = ml_dtypes.float8_e4m3
DR = mybir.MatmulPerfMode.DoubleRow

C = 192
HEADS = 8
CH = 24
W = 128
HOUT = 64
HIN = HOUT + 2
PXIN = HIN * W            # 8448
PXOUT = HOUT * W          # 8192
WS = 130                  # padded row stride in t buffers
LT = HIN * WS             # 8580
RS = 16                   # stripe out-rows
NS = HOUT // RS           # 4 stripes
MM = 512

TAPS = [(dy, dx) for dy in (0, 1, 2) for dx in (0, 1, 2)]
TOFF = [dy * WS + dx for dy, dx in TAPS]
PAIRS = [(0, 1), (2, 3), (4, 5), (6, 7), (8, 8)]

_CACHE = {}


def _chunks(total, step):
    out, s = [], 0
    while s < total:
        out.append((s, min(step, total - s)))
        s += step
    return out


def _mk(base_ap, off, dims):
    ap0 = [list(base_ap.ap[0])]
    return AP(base_ap.tensor, base_ap.offset + off,
              ap0 + [list(d) for d in dims])


def build_program():
    nc = bacc.Bacc("TRN2", target_bir_lowering=False, debug=False,
                   enable_asserts=False, num_devices=8)
    io = {}
    io["y8"] = nc.dram_tensor("y8", [128, 2 * PXIN], FP8,
                              kind="ExternalInput").ap()
    io["x4"] = nc.dram_tensor("x4", [128, 4 * PXIN], FP8,
                              kind="ExternalInput").ap()
    io["wqk8"] = nc.dram_tensor("wqk8", [128, 2 * 448], FP8,
                                kind="ExternalInput").ap()
    io["wv8"] = nc.dram_tensor("wv8", [128, 2 * 192], FP8,
                               kind="ExternalInput").ap()
    io["wve8"] = nc.dram_tensor("wve8", [128, 2 * 192], FP8,
                                kind="ExternalInput").ap()
    io["dgqk"] = nc.dram_tensor("dgqk", [128, 3 * 5 * 256], FP8,
                                kind="ExternalInput").ap()
    io["dgva"] = nc.dram_tensor("dgva", [128, 9 * 128], BF16,
                                kind="ExternalInput").ap()
    io["dvb"] = nc.dram_tensor("dvb", [64, 9], F32,
                               kind="ExternalInput").ap()
    io["dgvb16"] = nc.dram_tensor("dgvb16", [64, 9 * 64], BF16,
                                  kind="ExternalInput").ap()
    io["wpa"] = nc.dram_tensor("wpa", [128, C], BF16,
                               kind="ExternalInput").ap()
    io["wpb"] = nc.dram_tensor("wpb", [64, C], BF16,
                               kind="ExternalInput").ap()
    io["em"] = nc.dram_tensor("em", [HEADS, C], BF16,
                              kind="ExternalInput").ap()
    io["emba"] = nc.dram_tensor("emba", [128, C], BF16,
                                kind="ExternalInput").ap()
    io["embb"] = nc.dram_tensor("embb", [64, C], BF16,
                                kind="ExternalInput").ap()
    io["eye"] = nc.dram_tensor("eye", [128, 128], F32,
                               kind="ExternalInput").ap()
    io["tmpq"] = nc.dram_tensor("tmpq", [128, 2], F32,
                                kind="ExternalInput").ap()
    io["outp"] = nc.dram_tensor("outp", [C, PXOUT], BF16,
                                kind="ExternalOutput").ap()

    with tile.TileContext(nc) as tc, contextlib.ExitStack() as es:
        _emit(nc, tc, io, es)
    nc.compile()
    return nc


def _emit(nc, tc, io, es):
    # ---------------- pools & persistent tiles ------------------------
    wpool = es.enter_context(tc.tile_pool(name="w", bufs=1))
    dgqk = wpool.tile([128, 3 * 5 * 256], FP8, tag="dgqk")
    dgva = wpool.tile([128, 9 * 128], BF16, tag="dgva")
    dvb_t = wpool.tile([64, 9], F32, tag="dvb")
    dgvb16 = wpool.tile([64, 9 * 64], BF16, tag="dgvb16")
    wpa = wpool.tile([128, C], BF16, tag="wpa")
    wpb = wpool.tile([64, C], BF16, tag="wpb")
    em_t = wpool.tile([HEADS, C], BF16, tag="em")
    emba = wpool.tile([128, C], BF16, tag="emba")
    embb = wpool.tile([64, C], BF16, tag="embb")
    eye_t = wpool.tile([128, 128], F32, tag="eye")
    tmpq_t = wpool.tile([128, 2], F32, tag="tmpq")

    tpool = es.enter_context(tc.tile_pool(name="t", bufs=1))
    t_blk = [tpool.tile([128, LT], FP8, tag=f"t{b}", name=f"t{b}")
             for b in range(3)]
    tva16 = tpool.tile([128, LT], BF16, tag="tva16")
    tvb16 = tpool.tile([64, LT], BF16, tag="tvb16")

    dwp = es.enter_context(tc.tile_pool(name="dw", bufs=2))
    stkp = es.enter_context(tc.tile_pool(name="stk", bufs=1))
    vp = es.enter_context(tc.tile_pool(name="v", bufs=1))
    v16a = vp.tile([128, PXOUT], BF16, tag="v16a")
    v16b = vp.tile([64, PXOUT], BF16, tag="v16b")
    vbtmp = vp.tile([64, RS * W], BF16, tag="vbtmp")
    small = es.enter_context(tc.tile_pool(name="sm", bufs=1))
    drm = es.enter_context(tc.tile_pool(name="drm", bufs=1, space="DRAM"))

    ines = contextlib.ExitStack()
    inpool = ines.enter_context(tc.tile_pool(name="inp", bufs=1))
    y8 = inpool.tile([128, 2 * PXIN], FP8, tag="y8")
    x4 = inpool.tile([128, 4 * PXIN], FP8, tag="x4")
    wqk8 = inpool.tile([128, 2 * 448], FP8, tag="wqk8")
    wv8 = inpool.tile([128, 2 * 192], FP8, tag="wv8")
    wve8 = inpool.tile([128, 2 * 192], FP8, tag="wve8")

    # conv weights first on the sync queue (gate the first matmuls)
    for nm, t in (("wqk8", wqk8), ("wv8", wv8), ("wve8", wve8),
                  ("dgqk", dgqk), ("dgva", dgva), ("dvb", dvb_t),
                  ("dgvb16", dgvb16)):
        nc.sync.dma_start(t[:], io[nm])
    # inputs: split by pixel range, both channel-halves per range (Pool q)
    for part in range(4):
        a, b = part * PXIN // 4, (part + 1) * PXIN // 4
        nc.gpsimd.dma_start(y8[:, a:b], io["y8"][:, a:b])
        nc.gpsimd.dma_start(y8[:, PXIN + a:PXIN + b],
                            io["y8"][:, PXIN + a:PXIN + b])
        for sec in range(4):
            o = sec * PXIN
            nc.gpsimd.dma_start(x4[:, o + a:o + b], io["x4"][:, o + a:o + b])
    for nm, t in (("wpa", wpa), ("wpb", wpb), ("em", em_t), ("eye", eye_t),
                  ("emba", emba), ("embb", embb), ("tmpq", tmpq_t)):
        nc.sync.dma_start(t[:], io[nm])

    # pad-column zeroing
    for t in t_blk:
        v = t[:].rearrange("p (r w) -> p r w", w=WS)
        nc.vector.memset(v[:, :, 0:1], 0.0)
        nc.vector.memset(v[:, :, 129:130], 0.0)
    v = tva16[:].rearrange("p (r w) -> p r w", w=WS)
    nc.vector.memset(v[:, :, 0:1], 0.0)
    nc.vector.memset(v[:, :, 129:130], 0.0)
    v = tvb16[:].rearrange("p (r w) -> p r w", w=WS)
    nc.vector.memset(v[:, :, 0:1], 0.0)
    nc.vector.memset(v[:, :, 129:130], 0.0)

    w2v = wqk8[:].rearrange("p (two m) -> p two m", two=2)
    wv8v = wv8[:].rearrange("p (two m) -> p two m", two=2)
    wve8v = wve8[:].rearrange("p (two m) -> p two m", two=2)

    def y8rhs(n0, n):
        return _mk(y8[:], n0, [[PXIN, 2], [1, n]])

    def x8rhs(n0, n, res=0):
        return _mk(x4[:], res * PXIN + n0, [[2 * PXIN, 2], [1, n]])

    cchunks = _chunks(PXIN, MM)
    gab_sb = small.tile([128, 640], F32, tag="gabsb")
    qn_part = small.tile([128, 3 * NS + 4], F32, tag="qnp")
    junk = small.tile([128, RS * W], BF16, tag="junk")
    tva3 = tva16[:].rearrange("p (r w) -> p r w", w=WS)
    tvb3 = tvb16[:].rearrange("p (r w) -> p r w", w=WS)

    # ============ fused pass: conv qk+v, DW, gram =====================
    with tc.tile_pool(name="pa", bufs=1, space="PSUM") as pa:
        gAB = pa.tile([128, 640], F32, tag="gAB")
        g1v = gAB[:, 0:384].rearrange("p (r c) -> p r c", c=128)
        g2v = gAB[:, 384:640].rearrange("p (r c) -> p r c", c=128)

        def conv_qk(ci):
            n0, n = cchunks[ci]
            r0, nr = n0 // W, n // W
            ps0 = pa.tile([128, MM], F32, tag="cv0", name=f"cv0_{ci}")
            ps1 = pa.tile([128, MM], F32, tag="cv1", name=f"cv1_{ci}")
            ps2 = pa.tile([128, MM], F32, tag="cv2", name=f"cv2_{ci}",
                          bufs=2)
            nc.tensor.matmul(ps0[:, 0:n], w2v[:, :, 0:128], y8rhs(n0, n),
                             start=True, stop=True, perf_mode=DR)
            # t1 = [k0:64 @ parts 0:64 ; q128:192 @ parts 64:128]
            nc.tensor.matmul(ps1[:, 0:n], w2v[:, :, 128:256], y8rhs(n0, n),
                             start=True, stop=False, perf_mode=DR)
            nc.tensor.matmul(ps1[0:64, 0:n], w2v[:, :, 256:320],
                             x8rhs(n0, n), start=False, stop=True,
                             perf_mode=DR, skip_group_check=True)
            nc.tensor.matmul(ps2[:, 0:n], w2v[:, :, 320:448], x8rhs(n0, n),
                             start=True, stop=True, perf_mode=DR)
            for b, ps, eng in ((0, ps0, 0), (1, ps1, 1), (2, ps2, 1)):
                dst = t_blk[b][:].rearrange("p (r w) -> p r w", w=WS)
                src = ps[:, 0:n].rearrange("p (r w) -> p r w", w=W)
                if eng == 0:
                    nc.scalar.copy(dst[:, r0:r0 + nr, 1:129], src)
                else:
                    nc.vector.tensor_copy(dst[:, r0:r0 + nr, 1:129], src)

        def conv_v(ci):
            n0, n = cchunks[ci]
            r0, nr = n0 // W, n // W
            psa = pa.tile([128, MM], F32, tag="cv0", name=f"cva_{ci}")
            for m0, m1 in ((0, 128),):
                nc.tensor.matmul(psa[:, 0:n], wv8v[:, :, m0:m1],
                                 x8rhs(n0, n), start=True, stop=False,
                                 perf_mode=DR)
                nc.tensor.matmul(psa[:, 0:n], wv8v[:, :, m0:m1],
                                 x8rhs(n0, n, 1), start=False, stop=False,
                                 perf_mode=DR)
                nc.tensor.matmul(psa[:, 0:n], wve8v[:, :, m0:m1],
                                 x8rhs(n0, n), start=False, stop=True,
                                 perf_mode=DR)
            srca = psa[:, 0:n].rearrange("p (r w) -> p r w", w=W)
            nc.scalar.copy(tva3[:, r0:r0 + nr, 1:129], srca)
            psb = pa.tile([128, MM], F32, tag="cv1", name=f"cvb_{ci}")
            nc.tensor.matmul(psb[0:64, 0:n], wv8v[:, :, 128:192],
                             x8rhs(n0, n), start=True, stop=False,
                             perf_mode=DR)
            nc.tensor.matmul(psb[0:64, 0:n], wv8v[:, :, 128:192],
                             x8rhs(n0, n, 1), start=False, stop=False,
                             perf_mode=DR)
            nc.tensor.matmul(psb[0:64, 0:n], wve8v[:, :, 128:192],
                             x8rhs(n0, n), start=False, stop=True,
                             perf_mode=DR)
            srcb = psb[0:64, 0:n].rearrange("p (r w) -> p r w", w=W)
            nc.vector.tensor_copy(tvb3[:, r0:r0 + nr, 1:129], srcb)

        def dw_qk(s):
            dwq = [dwp.tile([128, RS * W], BF16, tag=f"dwq{b}",
                            name=f"dwq{b}_{s}") for b in range(3)]
            for b in range(3):
                dgv = dgqk[:, b * 1280:(b + 1) * 1280]
                for c in range(4):
                    r0 = s * RS + c * 4
                    ps = pa.tile([128, MM], F32, tag="dwps", bufs=2,
                                 name=f"dwps{b}_{s}_{c}")
                    psv = ps[:].rearrange("p (r w) -> p r w", w=W)
                    for pi, (ta, tb) in enumerate(PAIRS):
                        d = TOFF[tb] - TOFF[ta]
                        lhsT = dgv[:, pi * 256:(pi + 1) * 256].rearrange(
                            "p (two m) -> p two m", two=2)
                        rhs = _mk(t_blk[b][:], TOFF[ta] + r0 * WS,
                                  [[d, 2], [WS, 4], [1, W]])
                        nc.tensor.matmul(psv, lhsT, rhs, start=(pi == 0),
                                         stop=(pi == 4), perf_mode=DR)
                    dst = dwq[b][:, c * MM:(c + 1) * MM]
                    if (b * 4 + c) % 3 == 0:
                        nc.vector.tensor_copy(dst, ps[:])
                    else:
                        nc.scalar.copy(dst, ps[:])
            return dwq

        def stripe_tr(s, dwq):
            stk = stkp.tile([128, 3 * RS * W], BF16, tag="stk",
                            name=f"stk_{s}", bufs=1)
            for b in range(3):
                dst = stk[:, b * 2048:(b + 1) * 2048].rearrange(
                    "p (n f) -> p n f", f=128)
                nc.sync.dma_start_transpose(dst, dwq[b][:])
            return stk

        def stripe_gram(s, dwq, stk, first, last):
            for i in range(RS):
                st = first and i == 0
                sp = last and i == RS - 1
                lhs0 = stk[:, i * 128:i * 128 + 128]
                lhs1 = stk[:, 2048 + i * 128:2048 + i * 128 + 128]
                rhs3 = _mk(stk[:], i * 128, [[2048, 3], [1, 128]])
                rhs2 = _mk(stk[:], 2048 + i * 128, [[2048, 2], [1, 128]])
                nc.tensor.matmul(g1v, lhs0, rhs3, start=st, stop=sp)
                nc.tensor.matmul(g2v, lhs1, rhs2, start=st, stop=sp)

        def stripe_sq(s, dwq):
            nc.scalar.activation(junk[:], dwq[2][:], ACTF.Square,
                                 accum_out=qn_part[:, s:s + 1])

        def dw_va(s):
            for c in range(4):
                r0 = s * RS + c * 4
                ps = pa.tile([128, MM], F32, tag="cv2", bufs=2,
                             name=f"dwva_{s}_{c}")
                psv = ps[:].rearrange("p (r w) -> p r w", w=W)
                for t in range(9):
                    lhsT = dgva[:, t * 128:(t + 1) * 128]
                    rhs = _mk(tva16[:], TOFF[t] + r0 * WS,
                              [[WS, 4], [1, W]])
                    nc.tensor.matmul(psv, lhsT, rhs, start=(t == 0),
                                     stop=(t == 8))
                dst = v16a[:, r0 * W:(r0 + 4) * W]
                if c % 2 == 0:
                    nc.scalar.copy(dst, ps[:])
                else:
                    nc.vector.tensor_copy(dst, ps[:])

        PE_VB_TAPS = (0, 2, 4, 6)
        DVE_VB_TAPS = (1, 3, 5, 7, 8)

        def dw_vb(s):
            # PE part: 4 taps as bf16 diag matmuls, per 4-row chunk
            for c in range(4):
                r0 = s * RS + c * 4
                ps = pa.tile([128, MM], F32, tag="dwps", bufs=2,
                             name=f"dwvb_{s}_{c}")
                psv = ps[0:64, :].rearrange("p (r w) -> p r w", w=W)
                for ti, t in enumerate(PE_VB_TAPS):
                    lhsT = dgvb16[:, t * 64:(t + 1) * 64]
                    rhs = _mk(tvb16[:], TOFF[t] + r0 * WS,
                              [[WS, 4], [1, W]])
                    nc.tensor.matmul(psv, lhsT, rhs, start=(ti == 0),
                                     stop=(ti == 3))
                dst = vbtmp[:, c * MM:(c + 1) * MM]
                nc.scalar.copy(dst, ps[0:64, :])
            # DVE part: 5 taps + merge with PE partial
            r0 = s * RS
            vb = v16b[:, r0 * W:(r0 + RS) * W]
            vbv = vb.rearrange("p (r w) -> p r w", w=W)
            prod = small.tile([64, RS * W], BF16, tag="vbprod",
                              name=f"vbp_{s}")
            prodv = prod[:].rearrange("p (r w) -> p r w", w=W)
            for ti, t in enumerate(DVE_VB_TAPS):
                dy, dx = TAPS[t]
                view = tvb3[:, r0 + dy:r0 + dy + RS, dx:dx + 128]
                sc = dvb_t[:, t:t + 1]
                if ti == 0:
                    nc.vector.tensor_scalar(vbv, view, sc, None, ALU.mult)
                else:
                    nc.vector.tensor_scalar(prodv, view, sc, None, ALU.mult)
                    nc.vector.tensor_tensor(vb, vb, prod[:], ALU.add)
            nc.vector.tensor_tensor(vb, vb, vbtmp[:], ALU.add)

        emitted = 0
        for ci in range(len(cchunks)):
            conv_qk(ci)
            while emitted < NS and (ci + 1) * 4 >= (emitted * RS + RS + 2):
                s = emitted
                dwq = dw_qk(s)
                stripe_sq(s, dwq)
                stk = stripe_tr(s, dwq)
                stripe_gram(s, dwq, stk, s == 0, s == NS - 1)
                emitted += 1
        assert emitted == NS
        nc.scalar.copy(gab_sb[:], gAB[:])

        # ---- norms + bounce + collective (still inside psum pool) ----
        mk1 = small.tile([128, 128], F32, tag="mk1")
        mk2 = small.tile([128, 128], F32, tag="mk2")
        nc.gpsimd.tensor_tensor(mk1[:], gab_sb[:, 0:128], eye_t[:], ALU.mult)
        nc.gpsimd.tensor_tensor(mk2[:], gab_sb[:, 384:512], eye_t[:],
                                ALU.mult)
        jk2 = small.tile([128, NS], F32, tag="jk2")
        jk3 = small.tile([128, 128], F32, tag="jk3")
        nc.scalar.activation(jk3[:], mk1[:], ACTF.Copy,
                             accum_out=qn_part[:, NS:NS + 1])
        nc.scalar.activation(jk3[:], mk2[:], ACTF.Copy,
                             accum_out=qn_part[:, NS + 1:NS + 2])
        nc.scalar.activation(jk2[:], qn_part[:, 0:NS], ACTF.Copy,
                             accum_out=qn_part[:, NS + 2:NS + 3])
        qred = qn_part[:, NS:NS + 1]          # qn 0:128
        d2 = qn_part[:, NS + 1:NS + 2]        # kn0:64 | qn128:192
        kred = qn_part[:, NS + 2:NS + 3]      # kn 64:192

        bnc_a = small.tile([128, 26], F32, tag="bnca")
        nc.vector.tensor_copy(bnc_a[:, 24:25], qred)
        bounce_in = drm.tile([C, 26], F32)
        bounce_out = drm.tile([2 * C, 26], F32)
        # head gram blocks, rows q0:128 (g1: r1 k0:64 at cols 128:192,
        # r2 k64:192 at cols 256:384 -> col c<64 -> 128+c ; c>=64 -> 192+c)
        for h in range(6):
            r0, r1 = h * CH, min((h + 1) * CH, 128)
            c0, c1 = h * CH, (h + 1) * CH
            if c1 <= 64:
                nc.sync.dma_start(bnc_a[r0:r1, 0:24],
                                  gab_sb[r0:r1, 128 + c0:128 + c1])
            elif c0 >= 64:
                nc.sync.dma_start(bnc_a[r0:r1, 0:24],
                                  gab_sb[r0:r1, 192 + c0:192 + c1])
            else:
                nc.sync.dma_start(bnc_a[r0:r1, 0:64 - c0],
                                  gab_sb[r0:r1, 128 + c0:192])
                nc.sync.dma_start(bnc_a[r0:r1, 64 - c0:24],
                                  gab_sb[r0:r1, 256:192 + c1])
        nc.sync.dma_start(bounce_in[0:128, 0:25], bnc_a[:, 0:25])
        # kn col 25: rows 0:64 <- d2[0:64]; rows 64:192 <- kred
        nc.scalar.dma_start(bounce_in[0:64, 25:26], d2[0:64])
        nc.scalar.dma_start(bounce_in[64:192, 25:26], kred)
        # q-tail norms col 24 rows 128:192 <- d2[64:128]
        nc.scalar.dma_start(bounce_in[128:192, 24:25], d2[64:128])
        # heads 5b,6,7: rows q-tail = g2 rows 64:128; r2 cols = 384:512
        nc.scalar.dma_start(bounce_in[128:144, 0:24], gab_sb[64:80, 568:592])
        nc.scalar.dma_start(bounce_in[144:168, 0:24], gab_sb[80:104, 592:616])
        nc.scalar.dma_start(bounce_in[168:192, 0:24],
                            gab_sb[104:128, 616:640])
        nc.gpsimd.collective_compute(
            "AllGather", ALU.bypass,
            replica_groups=[[0, 1], [2, 3], [4, 5], [6, 7]],
            ins=[bounce_in[:].opt()], outs=[bounce_out[:].opt()])

        # ---- v phase: fills the collective window ---------------------
        emitted_b = 0
        for ci in range(len(cchunks)):
            conv_v(ci)
            while emitted_b < NS and (ci + 1) * 4 >= \
                    (emitted_b * RS + RS + 2):
                s = emitted_b
                dw_va(s)
                dw_vb(s)
                emitted_b += 1
        assert emitted_b == NS

    ines.close()
    outsb = es.enter_context(tc.tile_pool(name="osb", bufs=2))

    # ================= glue + attn-proj ===============================
    with tc.tile_pool(name="pb", bufs=1, space="PSUM") as pb:
        cmp_a = small.tile([128, 26], F32, tag="cmpa")
        cmp_b = small.tile([64, 26], F32, tag="cmpb")
        tmp_a = small.tile([128, 26], F32, tag="tmpa")
        tmp_b = small.tile([64, 26], F32, tag="tmpb")
        nc.sync.dma_start(cmp_a[:], bounce_out[0:128, :])
        nc.sync.dma_start(tmp_a[:], bounce_out[192:320, :])
        nc.sync.dma_start(cmp_b[:], bounce_out[128:192, :])
        nc.sync.dma_start(tmp_b[:], bounce_out[320:384, :])
        nc.vector.tensor_tensor(cmp_a[:], cmp_a[:], tmp_a[:], ALU.add)
        nc.vector.tensor_tensor(cmp_b[:], cmp_b[:], tmp_b[:], ALU.add)

        kn8 = small.tile([HEADS, CH], F32, tag="kn8")
        kn8t = small.tile([HEADS, CH], F32, tag="kn8t")
        nc.sync.dma_start(
            kn8[:], bounce_out[0:192, :].rearrange(
                "(h c) k -> h c k", c=CH)[:, :, 25])
        nc.sync.dma_start(
            kn8t[:], bounce_out[192:384, :].rearrange(
                "(h c) k -> h c k", c=CH)[:, :, 25])
        nc.vector.tensor_tensor(kn8[:], kn8[:], kn8t[:], ALU.add)

        rq_a = small.tile([128, 3], F32, tag="rqa")
        rq_b = small.tile([64, 3], F32, tag="rqb")
        for ti, (cmp, rq, nrow) in enumerate(((cmp_a, rq_a, 128),
                                              (cmp_b, rq_b, 64))):
            nc.scalar.activation(rq[:, 0:1], cmp[:, 24:25], ACTF.Sqrt)
            nc.vector.reciprocal(rq[:, 1:2], rq[:, 0:1])
            nc.vector.tensor_scalar(rq[:, 2:3], rq[:, 1:2],
                                    tmpq_t[0:nrow, ti:ti + 1], None,
                                    ALU.mult)
        rk8 = small.tile([HEADS, 2 * CH], F32, tag="rk8")
        nc.scalar.activation(rk8[:, 0:CH], kn8[:], ACTF.Sqrt)
        nc.vector.reciprocal(rk8[:, CH:2 * CH], rk8[:, 0:CH])
        rk8b = small.tile([HEADS, CH], BF16, tag="rk8b")
        nc.vector.tensor_copy(rk8b[:], rk8[:, CH:2 * CH])

        knb_a = small.tile([128, CH], F32, tag="knba")
        knb_b = small.tile([64, CH], F32, tag="knbb")
        knb_ps = pb.tile([128, MM], F32, tag="pja", name="knb_ps", bufs=2)
        nc.tensor.matmul(knb_ps[:, 0:CH], em_t[:, 0:128], rk8b[:],
                         start=True, stop=True)
        nc.scalar.copy(knb_a[:], knb_ps[:, 0:CH])
        knb_ps2 = pb.tile([128, MM], F32, tag="pja", name="knb_ps2", bufs=2)
        nc.tensor.matmul(knb_ps2[0:64, 0:CH], em_t[:, 128:192], rk8b[:],
                         start=True, stop=True)
        nc.scalar.copy(knb_b[:], knb_ps2[0:64, 0:CH])

        attn16 = small.tile([128, CH], BF16, tag="att16a")
        attn16b = small.tile([64, CH], BF16, tag="att16b")
        for cmp, rq, knb, a16, nrow in ((cmp_a, rq_a, knb_a, attn16, 128),
                                        (cmp_b, rq_b, knb_b, attn16b, 64)):
            at = small.tile([128, CH], F32, tag="atf")
            sm = small.tile([128, 4], F32, tag="smx")
            nc.vector.tensor_scalar(at[0:nrow, :], cmp[0:nrow, 0:CH],
                                    rq[:, 2:3], None, ALU.mult)
            nc.vector.tensor_tensor(at[0:nrow, :], at[0:nrow, :], knb[:],
                                    ALU.mult)
            nc.vector.tensor_reduce(sm[0:nrow, 0:1], at[0:nrow, :], AX.X,
                                    ALU.max)
            nc.vector.tensor_scalar(at[0:nrow, :], at[0:nrow, :],
                                    sm[0:nrow, 0:1], None, ALU.subtract)
            nc.scalar.activation(at[0:nrow, :], at[0:nrow, :], ACTF.Exp)
            nc.vector.tensor_reduce(sm[0:nrow, 1:2], at[0:nrow, :], AX.X,
                                    ALU.add)
            nc.vector.reciprocal(sm[0:nrow, 2:3], sm[0:nrow, 1:2])
            nc.vector.tensor_scalar(a16[0:nrow, :], at[0:nrow, :],
                                    sm[0:nrow, 2:3], None, ALU.mult)

        # BD via stride-0 head-repeat x mask
        bd_a = small.tile([128, C], BF16, tag="bda")
        bd_b = small.tile([64, C], BF16, tag="bdb")
        rep_a = _mk(attn16[:], 0, [[0, HEADS], [1, CH]])
        rep_b = _mk(attn16b[:], 0, [[0, HEADS], [1, CH]])
        nc.vector.tensor_tensor(
            bd_a[:].rearrange("p (h c) -> p h c", c=CH), rep_a,
            emba[:].rearrange("p (h c) -> p h c", c=CH), ALU.mult)
        nc.vector.tensor_tensor(
            bd_b[:].rearrange("p (h c) -> p h c", c=CH), rep_b,
            embb[:].rearrange("p (h c) -> p h c", c=CH), ALU.mult)

        wpp_a16 = small.tile([128, C], BF16, tag="wppa")
        wpp_b16 = small.tile([64, C], BF16, tag="wppb")
        wpp_ps = pb.tile([128, MM], F32, tag="pjb", name="wpp_ps", bufs=2)
        nc.tensor.matmul(wpp_ps[:, 0:C], bd_a[:, 0:128], wpa[:],
                         start=True, stop=False)
        nc.tensor.matmul(wpp_ps[:, 0:C], bd_b[:, 0:128], wpb[:],
                         start=False, stop=True)
        nc.scalar.copy(wpp_a16[:], wpp_ps[:, 0:C])
        wpp_ps2 = pb.tile([128, MM], F32, tag="pjb", name="wpp_ps2", bufs=2)
        nc.tensor.matmul(wpp_ps2[0:64, 0:C], bd_a[:, 128:192], wpa[:],
                         start=True, stop=False)
        nc.tensor.matmul(wpp_ps2[0:64, 0:C], bd_b[:, 128:192], wpb[:],
                         start=False, stop=True)
        nc.scalar.copy(wpp_b16[:], wpp_ps2[0:64, 0:C])

        for g in range(4):
            oa = outsb.tile([128, 2048], BF16, tag="oa", name=f"oa_{g}")
            ob = outsb.tile([64, 2048], BF16, tag="ob", name=f"ob_{g}")
            for cc in range(4):
                n0 = g * 2048 + cc * MM
                pja = pb.tile([128, MM], F32, tag="pja", bufs=2,
                              name=f"pja_{g}_{cc}")
                pjb = pb.tile([64, MM], F32, tag="pjb", bufs=2,
                              name=f"pjb_{g}_{cc}")
                nc.tensor.matmul(pja[:], wpp_a16[:, 0:128],
                                 v16a[:, n0:n0 + MM], start=True, stop=False)
                nc.tensor.matmul(pja[:], wpp_b16[:, 0:128],
                                 v16b[:, n0:n0 + MM], start=False, stop=True)
                nc.tensor.matmul(pjb[:], wpp_a16[:, 128:192],
                                 v16a[:, n0:n0 + MM], start=True, stop=False)
                nc.tensor.matmul(pjb[:], wpp_b16[:, 128:192],
                                 v16b[:, n0:n0 + MM], start=False, stop=True)
                nc.scalar.copy(oa[:, cc * MM:(cc + 1) * MM], pja[:])
                nc.vector.tensor_copy(ob[:, cc * MM:(cc + 1) * MM], pjb[:])
            nc.scalar.dma_start(io["outp"][0:128, g * 2048:(g + 1) * 2048],
                                oa[:])
            nc.scalar.dma_start(io["outp"][128:192, g * 2048:(g + 1) * 2048],
                                ob[:])


# ======================================================================
def _interleave2(w, cols):
    out = np.zeros((128, 2, cols), np.float32)
    out[:, 0, :] = w[0:128]
    out[0:64, 1, :] = w[128:192]
    return out


def _diag_pair_block(wcols, mw):
    npair = len(wcols) // 2
    out = np.zeros((mw, npair, 2, mw), np.float32)
    idx = np.arange(mw)
    for p in range(npair):
        out[idx, p, 0, idx] = wcols[2 * p]
        out[idx, p, 1, idx] = wcols[2 * p + 1]
    return out.reshape(mw, npair * 2 * mw)


def _prep_inputs(x, y, qkv_w, dw_w, proj_w, temperature):
    f8 = lambda a: a.astype(FP8NP)
    f8v = lambda a: a.astype(FP8NP).astype(np.float32)

    WqT = np.ascontiguousarray(qkv_w[0:C].T)
    WkT = np.ascontiguousarray(qkv_w[C:2 * C].T)
    WvT = np.ascontiguousarray(qkv_w[2 * C:3 * C].T)

    wqk = np.zeros((128, 2, 448), np.float32)
    wqk[:, :, 0:128] = _interleave2(WqT, C)[:, :, 0:128]
    wqk[:, :, 192:256] = _interleave2(WqT, C)[:, :, 128:192]
    wqk[:, :, 256:320] = _interleave2(WkT, C)[:, :, 0:64]
    wqk[:, :, 320:448] = _interleave2(WkT, C)[:, :, 64:192]
    wqk8 = f8(wqk.reshape(128, 2 * 448))

    wv = _interleave2(WvT, C)
    wv8 = f8(wv)
    wve8 = f8(wv - wv8.astype(np.float32))

    dw = dw_w.reshape(3 * C, 9).astype(np.float32)
    dw_q, dw_k, dw_v = dw[0:C], dw[C:2 * C], dw[2 * C:3 * C]
    blocks = [dw_q[0:128],
              np.concatenate([dw_k[0:64], dw_q[128:192]], axis=0),
              dw_k[64:192]]
    dgqk = np.zeros((128, 3, 5 * 256), np.float32)
    for b, blk in enumerate(blocks):
        cols = [blk[:, t] for t in range(9)]
        cols.append(np.zeros(128, np.float32))
        dgqk[:, b, :] = _diag_pair_block(cols, 128)
    dgqk8 = f8(dgqk.reshape(128, 3 * 5 * 256))

    dva = dw_v[0:128].astype(np.float32)
    dgva = np.zeros((128, 9, 128), np.float32)
    idx = np.arange(128)
    for t in range(9):
        dgva[idx, t, idx] = dva[:, t]
    dgva = dgva.reshape(128, 9 * 128).astype(BF16NP)
    dvb = np.ascontiguousarray(dw_v[128:192].astype(np.float32))
    dgvb16 = np.zeros((64, 9, 64), np.float32)
    idx64 = np.arange(64)
    for t in range(9):
        dgvb16[idx64, t, idx64] = dvb[:, t]
    dgvb16 = dgvb16.reshape(64, 9 * 64).astype(BF16NP)

    WpT = np.ascontiguousarray(proj_w.T).astype(np.float32)
    wpa = WpT[0:128].astype(BF16NP)
    wpb = WpT[128:192].astype(BF16NP)
    tmpq_full = np.repeat(np.asarray(temperature, np.float32).reshape(HEADS),
                          CH)
    tmpq = np.zeros((128, 2), np.float32)
    tmpq[:, 0] = tmpq_full[0:128]
    tmpq[0:64, 1] = tmpq_full[128:192]
    em = np.zeros((HEADS, C), np.float32)
    for hh in range(HEADS):
        em[hh, hh * CH:(hh + 1) * CH] = 1.0
    emb = np.zeros((C, C), np.float32)
    for cc in range(C):
        hh = cc // CH
        emb[cc, hh * CH:(hh + 1) * CH] = 1.0

    in_maps = []
    for core in range(8):
        bi, half = core // 2, core % 2
        r0 = half * HOUT - 1
        xsl = np.zeros((C, HIN, W), np.float32)
        ysl = np.zeros((C, HIN, W), np.float32)
        lo, hi = max(r0, 0), min(r0 + HIN, 2 * HOUT)
        xsl[:, lo - r0:hi - r0] = x[bi, :, lo:hi]
        ysl[:, lo - r0:hi - r0] = y[bi, :, lo:hi]
        xf = xsl.reshape(C, PXIN)
        yf = ysl.reshape(C, PXIN)
        x8 = xf.astype(FP8NP).astype(np.float32)
        xe8 = f8(xf - x8)
        x4 = np.zeros((128, 2, 2, PXIN), FP8NP)
        x4[:, 0, 0, :] = f8(x8[0:128])
        x4[0:64, 1, 0, :] = f8(x8[128:192])
        x4[:, 0, 1, :] = xe8[0:128]
        x4[0:64, 1, 1, :] = xe8[128:192]
        y8 = np.zeros((128, 2, PXIN), FP8NP)
        y8[:, 0, :] = f8(yf[0:128])
        y8[0:64, 1, :] = f8(yf[128:192])
        in_maps.append({
            "y8": y8.reshape(128, 2 * PXIN),
            "x4": x4.reshape(128, 4 * PXIN),
            "wqk8": wqk8, "wv8": f8(wv8.reshape(128, 2 * 192)),
            "wve8": wve8.reshape(128, 2 * 192),
            "dgqk": dgqk8, "dgva": dgva, "dvb": dvb, "dgvb16": dgvb16,
            "wpa": wpa, "wpb": wpb, "em": em.astype(BF16NP),
            "emba": emb[0:128].astype(BF16NP),
            "embb": emb[128:192].astype(BF16NP),
            "eye": np.eye(128, dtype=np.float32), "tmpq": tmpq,
        })
    return in_maps


def kernel(x, y, qkv_w, dw_w, proj_w, temperature, _trace=False):
    x = np.asarray(x, np.float32)
    y = np.asarray(y, np.float32)
    if "nc" not in _CACHE:
        _CACHE["nc"] = build_program()
    nc = _CACHE["nc"]
    in_maps = _prep_inputs(x, y, np.asarray(qkv_w, np.float32),
                           np.asarray(dw_w, np.float32),
                           np.asarray(proj_w, np.float32),
                           np.asarray(temperature, np.float32))
    res = bass_utils.run_bass_kernel_spmd(nc, in_maps,
                                          core_ids=list(range(8)),
                                          trace=_trace)
    _CACHE["last_result"] = res
    out = np.empty((4, C, 2 * HOUT, W), np.float32)
    for core in range(8):
        bi, half = core // 2, core % 2
        out[bi, :, half * HOUT:(half + 1) * HOUT] = \
            res.results[core]["outp"].astype(np.float32).reshape(C, HOUT, W)
    return out
